# revision 44
# baseline (speedup 1.0000x reference)
"""Multi-head causal attention with RoPE on 8 Trainium2 NeuronCores.

Sharding: data-parallel over batch (2) x tensor-parallel over heads (16 -> 4
per core). Each core computes q/k/v projections for its 4 heads on its batch
element, attention, and a partial output projection (its rows of wo); the
host sums the 4 partials per batch element.

Device-side layout: everything is computed "transposed" (scores held as
[t, s]) so no on-device transposes are needed anywhere; softmax denominators
come from an all-ones matmul (partition reduction on the tensor engine).
RoPE pair-swap is done by permuting the wq/wk columns on the host into
(even|odd) half-layout so the swap becomes two partition-halved SBUF->SBUF
DMA copies.

Speed scheme (causal variant): hybrid precision. The first sequence chunk
(queries s<512 / keys t<512, where causal attention is concentrated and
quantization errors are amplified) runs in bf16; chunks 1-3 run their
q/k/v projections, attn@v, and output projection as fp8e4m3 DoubleRow
matmuls (2x tensor-engine throughput; two 128-deep K-tiles per pass).
Scores and softmax-denominator matmuls stay bf16 everywhere. The causal
diagonal mask is applied by zeroing exp tiles with affine_select on the
(otherwise idle) Pool engine instead of identity-matmul mask adds on the
PE. exp uses a -1 bias shift so fp8 exp tiles cannot overflow (max logit
~6.0 -> e^5.0 = 148 < 240); the shift cancels in the softmax ratio.
"""

import math

import ml_dtypes
import numpy as np

import concourse.bass as bass
import concourse.mybir as mybir
import concourse.tile as tile
from concourse import bacc
from concourse.bass_utils import run_bass_kernel_spmd

BF16 = ml_dtypes.bfloat16
FP8NP = ml_dtypes.float8_e4m3
F32 = mybir.dt.float32
BF = mybir.dt.bfloat16
FP8 = mybir.dt.float8e4
AF = mybir.ActivationFunctionType
DRMODE = mybir.MatmulPerfMode.DoubleRow

N_CORES = 8
B = 2
S = 2048
D = 2048
H = 16
HD = 128
H_LOC = 4          # heads per core
N_LOC = H_LOC * HD  # 512 local head dims
NJ = 4             # s-chunks
SC = S // NJ       # 512 s-chunk width
DCH = D // 128     # 16 contraction chunks
SCALE = 1.0 / math.sqrt(HD)
EXP_BIAS = -1.0    # exp(SCALE*score - 1): keeps fp8 exp tiles under 240

_BUILDS: dict = {}
LAST_RESULT = None


def _build_causal(nj: int = NJ):
    nc = bacc.Bacc("TRN2", target_bir_lowering=False, debug=False,
                   num_devices=N_CORES)

    # chunk 0 inputs (bf16)
    xt0_d = nc.dram_tensor("xt0", [128, DCH, SC], BF, kind="ExternalInput").ap()
    wqkb_d = nc.dram_tensor("wqkb", [2, H_LOC, 128, DCH, 128], BF,
                            kind="ExternalInput").ap()  # head-major q|k bf16
    wvb_d = nc.dram_tensor("wvb", [128, DCH, N_LOC], BF, kind="ExternalInput").ap()
    wob_d = nc.dram_tensor("wob", [128, H_LOC, D], BF, kind="ExternalInput").ap()
    # chunks 1-3 inputs (fp8)
    xt8_d = nc.dram_tensor("xt8", [NJ - 1, 128, DCH, SC], FP8,
                           kind="ExternalInput").ap()
    wq8_d = nc.dram_tensor("wq8", [128, DCH, N_LOC], FP8, kind="ExternalInput").ap()
    wk8_d = nc.dram_tensor("wk8", [128, DCH, N_LOC], FP8, kind="ExternalInput").ap()
    wv8_d = nc.dram_tensor("wv8", [128, DCH, N_LOC], FP8, kind="ExternalInput").ap()
    wo8_d = nc.dram_tensor("wo8", [128, H_LOC, D], FP8, kind="ExternalInput").ap()
    cose_d = nc.dram_tensor("cose", [128, S], F32, kind="ExternalInput").ap()
    sine_d = nc.dram_tensor("sine", [128, S], F32, kind="ExternalInput").ap()
    out_d = nc.dram_tensor("out", [S, D], BF, kind="ExternalOutput").ap()
    out_v = out_d.rearrange("(a p) d -> a p d", p=128)

    with tile.TileContext(nc) as tc:
        with (
            tc.tile_pool(name="singles", bufs=1) as singles,
            tc.tile_pool(name="doubles", bufs=2) as doubles,
            tc.tile_pool(name="triples", bufs=3) as triples,
            tc.tile_pool(name="quads", bufs=4) as quads,
            tc.tile_pool(name="hexes", bufs=7) as hexes,
            tc.tile_pool(name="ps1", bufs=1, space="PSUM") as ps1,
            tc.tile_pool(name="ps2", bufs=2, space="PSUM") as ps2,
        ):
            # ---- persistent tensors ----
            # chunk-0 q/k weights are streamed per head from wqkb_d; wv/wo
            # bf16 stay resident (v uses full-width rhs; wo reused per dc).
            wvb_sb = singles.tile([128, DCH, N_LOC], BF, tag="wvb")
            wob_sb = singles.tile([128, H_LOC, D], BF, tag="wob")
            wq8_sb = singles.tile([128, DCH, N_LOC], FP8, tag="wq8")
            wk8_sb = singles.tile([128, DCH, N_LOC], FP8, tag="wk8")
            wv8_sb = singles.tile([128, DCH, N_LOC], FP8, tag="wv8")
            wo8_sb = singles.tile([128, H_LOC, D], FP8, tag="wo8")
            xt0_sb = singles.tile([128, DCH, SC], BF, tag="xt0")
            # startup DMA order: xt0 pieces on the gpsimd queue, q/k weight
            # streams (piecewise) on sync in consumption order, cos/sin on
            # the scalar queue, later-needed tensors behind.
            for q4 in range(4):
                nc.gpsimd.dma_start(out=xt0_sb[:, 4 * q4:4 * (q4 + 1), :],
                                    in_=xt0_d[:, 4 * q4:4 * (q4 + 1), :])
            wqk_sb = {}
            for wi in range(2):
                for h in range(H_LOC):
                    wt_tile = triples.tile([128, DCH, 128], BF, tag="wstream")
                    for piece in range(2):
                        nc.sync.dma_start(
                            out=wt_tile[:, 8 * piece:8 * (piece + 1), :],
                            in_=wqkb_d[wi, h][:, 8 * piece:8 * (piece + 1), :])
                    wqk_sb[(wi, h)] = wt_tile
            cose_sb = singles.tile([128, S], F32, tag="cose")
            sine_sb = singles.tile([128, S], F32, tag="sine")
            nc.scalar.dma_start(out=cose_sb[:], in_=cose_d[:])
            nc.scalar.dma_start(out=sine_sb[:], in_=sine_d[:])
            nc.sync.dma_start(out=wvb_sb[:], in_=wvb_d[:])
            nc.sync.dma_start(out=wq8_sb[:], in_=wq8_d[:])
            nc.sync.dma_start(out=wk8_sb[:], in_=wk8_d[:])
            nc.sync.dma_start(out=wv8_sb[:], in_=wv8_d[:])
            nc.sync.dma_start(out=wob_sb[:], in_=wob_d[:])
            nc.sync.dma_start(out=wo8_sb[:], in_=wo8_d[:])
            ones_sb = singles.tile([128, 128], BF, tag="ones")
            nc.vector.memset(ones_sb[:], 1.0)
            bias_sb = singles.tile([128, 1], F32, tag="bias")
            nc.vector.memset(bias_sb[:], EXP_BIAS)
            # k^T (rotated, bf16) accumulates across chunks; v in fp8 for
            # DR pv (all chunks) + bf16 copy of t-chunk 0 for chunk-0 pv
            ktrot = singles.tile([128, H_LOC, S], BF, tag="ktrot")
            v8_sb = singles.tile([128, NJ * H_LOC, SC], FP8, tag="v8")
            vbf_sb = singles.tile([128, H_LOC, SC], BF, tag="vbf")

            def qk_unit(j, xt_sb, wi, hp, qdest):
                """DR/bf16 projections + rope for heads hp*2, hp*2+1 of
                q (wi=0) or k (wi=1) of chunk j."""
                js = j * SC
                dest, dsl = ((qdest, None) if wi == 0 else
                             (ktrot, slice(js, js + SC)))
                w8_sb = wq8_sb if wi == 0 else wk8_sb
                parts = []
                for hh in range(2):
                    h = hp * 2 + hh
                    if j == 0 and hp == 1:
                        ps = ps1.tile([128, SC], F32,
                                      tag="pv" if hh else "sums")
                    else:
                        ps = ps2.tile([128, SC], F32, tag="qkv1")
                    if j == 0:
                        wt = wqk_sb[(wi, h)]
                        for d in range(DCH):
                            nc.tensor.matmul(
                                ps[:], wt[:, d, :], xt_sb[:, d, :],
                                start=(d == 0), stop=(d == DCH - 1),
                            )
                    else:
                        for d in range(DCH // 2):
                            nc.tensor.matmul(
                                ps[:],
                                w8_sb[:, 2 * d:2 * d + 2,
                                      h * 128:(h + 1) * 128],
                                xt_sb[:, 2 * d:2 * d + 2, :],
                                start=(d == 0),
                                stop=(d == DCH // 2 - 1),
                                perf_mode=DRMODE,
                            )
                    a_sb = quads.tile([128, SC], F32, tag="ropeA")
                    nc.vector.tensor_mul(
                        a_sb[:], ps[:], cose_sb[:, js:js + SC])
                    b_sb = triples.tile([128, SC], F32, tag="ropeB")
                    nc.vector.tensor_mul(
                        b_sb[:], ps[:], sine_sb[:, js:js + SC])
                    # half-swap via SBUF->SBUF DMA (cross-partition)
                    b2_sb = triples.tile([128, SC], F32, tag="ropeB2")
                    nc.scalar.dma_start(out=b2_sb[0:64, :],
                                        in_=b_sb[64:128, :])
                    nc.scalar.dma_start(out=b2_sb[64:128, :],
                                        in_=b_sb[0:64, :])
                    parts.append((h, a_sb, b2_sb))
                for h, a_sb, b2_sb in parts:
                    if dsl is None:
                        dst = dest[:, h, :]
                    else:
                        dst = dest[:, h, dsl]
                    nc.vector.tensor_add(dst, a_sb[:], b2_sb[:])

            def v_unit(j, xt_sb, tl):
                ps = ps2.tile([128, SC], F32, tag="qkv1")
                if j == 0:
                    for d in range(DCH):
                        nc.tensor.matmul(
                            ps[:],
                            xt_sb[:, d, tl * 128:(tl + 1) * 128],
                            wvb_sb[:, d, :],
                            start=(d == 0), stop=(d == DCH - 1),
                        )
                    nc.scalar.copy(out=vbf_sb[:, tl, :], in_=ps[:])
                    # pool can't read PSUM; mirror to fp8 from the SBUF copy
                    nc.gpsimd.tensor_copy(v8_sb[:, tl, :], vbf_sb[:, tl, :])
                else:
                    for d in range(DCH // 2):
                        nc.tensor.matmul(
                            ps[:],
                            xt_sb[:, 2 * d:2 * d + 2,
                                  tl * 128:(tl + 1) * 128],
                            wv8_sb[:, 2 * d:2 * d + 2, :],
                            start=(d == 0), stop=(d == DCH // 2 - 1),
                            perf_mode=DRMODE,
                        )
                    nc.scalar.copy(out=v8_sb[:, 4 * j + tl, :], in_=ps[:])

            def projection_units(j, qdest):
                """Emit the xt DMA now; return per-unit closures for the
                matmul/rope work (interleaved into the previous chunk's
                attention as PE bubble fillers)."""
                # sync queue (idle after startup): the gpsimd queue would
                # serialize these triggers behind pool-engine compute
                xt_sb = doubles.tile([128, DCH, SC], FP8, tag="xt8")
                for q4 in range(4):
                    nc.sync.dma_start(
                        out=xt_sb[:, 4 * q4:4 * (q4 + 1), :],
                        in_=xt8_d[j - 1][:, 4 * q4:4 * (q4 + 1), :])
                units = []
                for wi in range(2):
                    for hp in range(2):
                        units.append(lambda wi=wi, hp=hp:
                                     qk_unit(j, xt_sb, wi, hp, qdest))
                for tl in range(4):
                    units.append(lambda tl=tl: v_unit(j, xt_sb, tl))
                return units

            def projections0(qdest):
                for wi in range(2):
                    for hp in range(2):
                        qk_unit(0, xt0_sb, wi, hp, qdest)
                for tl in range(4):
                    v_unit(0, xt0_sb, tl)

            def wo_units(j, attnT_j, st, dcs):
                for dc in dcs:
                    wps = ps2.tile([128, SC], F32, tag="qkv1")
                    if j == 0:
                        for h2 in range(H_LOC):
                            nc.tensor.matmul(
                                wps[:],
                                attnT_j[:, h2, st * 128:(st + 1) * 128],
                                wob_sb[:, h2, dc * SC:(dc + 1) * SC],
                                start=(h2 == 0), stop=(h2 == H_LOC - 1),
                            )
                    else:
                        for hp in range(H_LOC // 2):
                            nc.tensor.matmul(
                                wps[:],
                                attnT_j[:, 2 * hp:2 * hp + 2,
                                        st * 128:(st + 1) * 128],
                                wo8_sb[:, 2 * hp:2 * hp + 2,
                                       dc * SC:(dc + 1) * SC],
                                start=(hp == 0), stop=(hp == H_LOC // 2 - 1),
                                perf_mode=DRMODE,
                            )
                    o_sb = triples.tile([128, SC], BF, tag="ostage")
                    if (st + dc) % 2 == 0:
                        nc.scalar.copy(out=o_sb[:], in_=wps[:])
                    else:
                        nc.vector.tensor_copy(o_sb[:], wps[:])
                    nc.sync.dma_start(
                        out=out_v[4 * j + st][:, dc * SC:(dc + 1) * SC],
                        in_=o_sb[:])

            def attention_and_wo(j, qtrot, fillers=(), self_wo=False):
                fillers = list(fillers)
                emitted = [0]

                def pump(frac):
                    # keep the in-order PE queue fed: emit filler units up to
                    # the given fraction of attention progress
                    n_emit = min(len(fillers),
                                 int(len(fillers) * frac + 0.999))
                    while emitted[0] < n_emit:
                        fillers[emitted[0]]()
                        emitted[0] += 1

                bf = j == 0
                edt = BF if bf else FP8
                if bf:
                    attnT_j = singles.tile([128, H_LOC, SC], BF, tag="attnT_bf")
                else:
                    attnT_j = doubles.tile([128, H_LOC, SC], FP8, tag="attnT8")
                # diagonal pair-groups first: their longer select+exp chain
                # overlaps the remaining full groups' matmuls
                pg_order = list(range(2 * j, 2 * j + 2)) + list(range(2 * j))
                ng = len(pg_order)
                for h in range(H_LOC):
                    sums_ps = ps1.tile([128, SC], F32, tag="sums")
                    pv_ps = ps1.tile([128, SC], F32, tag="pv")

                    def emit_scores_exp(gi):
                        """scores + exp/select/epair chain for group gi;
                        returns (exp_sb, equad-or-None)."""
                        pg = pg_order[gi]
                        sc_ps = ps2.tile([128, 2, SC], F32, tag="sc")
                        if bf:
                            exp_sb = doubles.tile([128, 2, SC], BF,
                                                  tag="exp_bf")
                        else:
                            exp_sb = triples.tile([128, 2, SC], FP8, tag="exp8")
                        diag = pg >= 2 * j
                        for i_ in range(2):
                            tt = pg * 2 + i_
                            if diag:
                                # columns [0, 128p) of a diagonal block are
                                # fully causal-masked: skip computing them
                                p = tt - 4 * j
                                w0 = 128 * p
                            else:
                                w0 = 0
                            nc.tensor.matmul(
                                sc_ps[:, i_, w0:SC],
                                ktrot[:, h, tt * 128:(tt + 1) * 128],
                                qtrot[:, h, w0:SC],
                                start=True, stop=True,
                            )
                        if diag:
                            # skipped columns are zeroed by a dependency-free
                            # memset (issued before exp); the select then only
                            # covers the block's own 128-wide partial triangle
                            for i_ in range(2):
                                p = pg * 2 + i_ - 4 * j
                                if p > 0:
                                    nc.gpsimd.memset(
                                        exp_sb[:, i_, 0:128 * p], 0.0)
                                nc.scalar.activation(
                                    out=exp_sb[:, i_, 128 * p:SC],
                                    in_=sc_ps[:, i_, 128 * p:SC],
                                    func=AF.Exp, scale=SCALE, bias=bias_sb[:])
                                nc.gpsimd.affine_select(
                                    exp_sb[:, i_, 128 * p:128 * (p + 1)],
                                    exp_sb[:, i_, 128 * p:128 * (p + 1)],
                                    pattern=[[1, 128]],
                                    compare_op=mybir.AluOpType.is_ge,
                                    fill=0.0, base=0,
                                    channel_multiplier=-1)
                        else:
                            nc.scalar.activation(out=exp_sb[:], in_=sc_ps[:],
                                                 func=AF.Exp, scale=SCALE,
                                                 bias=bias_sb[:])
                        epair = doubles.tile([128, SC], BF, tag="epair")
                        nc.vector.tensor_add(epair[:], exp_sb[:, 0, :],
                                             exp_sb[:, 1, :])
                        equad = None
                        if gi % 2 == 0:
                            pend_epair[0] = epair
                        else:
                            equad = triples.tile([128, SC], BF, tag="equad")
                            nc.vector.tensor_add(equad[:], pend_epair[0][:],
                                                 epair[:])
                        return exp_sb, equad

                    def emit_pe_consumers(gi, exp_sb, equad):
                        pg = pg_order[gi]
                        if equad is not None:
                            nc.tensor.matmul(sums_ps[:], ones_sb[:], equad[:],
                                             start=gi == 1, stop=gi == ng - 1)
                        if bf:
                            for i_ in range(2):
                                tt = pg * 2 + i_
                                nc.tensor.matmul(
                                    pv_ps[:],
                                    vbf_sb[:, tt, h * 128:(h + 1) * 128],
                                    exp_sb[:, i_, :],
                                    start=gi == 0 and i_ == 0,
                                    stop=gi == ng - 1 and i_ == 1)
                        else:
                            nc.tensor.matmul(
                                pv_ps[:],
                                v8_sb[:, 2 * pg:2 * pg + 2,
                                      h * 128:(h + 1) * 128],
                                exp_sb[:, :, :],
                                start=gi == 0, stop=gi == ng - 1,
                                perf_mode=DRMODE)

                    # depth-1 software pipeline: group gi's scores run on the
                    # PE while group gi-1's exp/select chain completes, so the
                    # ones/pv consumers of gi-1 issue with their deps resolved
                    pend_epair = [None]
                    pending_grp = None
                    for gi in range(ng):
                        made = emit_scores_exp(gi)
                        if pending_grp is not None:
                            emit_pe_consumers(*pending_grp)
                        pending_grp = (gi, *made)
                    emit_pe_consumers(*pending_grp)
                    recip_sb = doubles.tile([128, SC], F32, tag="recip")
                    nc.vector.reciprocal_approx_fast(out=recip_sb[:],
                                                     in_=sums_ps[:])
                    nc.vector.tensor_mul(attnT_j[:, h, :], pv_ps[:],
                                         recip_sb[:])
                    pump((h + 1) / H_LOC if h < H_LOC - 1 else 1.0)
                return attnT_j

            # prev-chunk wo and next-chunk projections run interleaved with
            # each chunk's attention, keeping the in-order PE queue fed while
            # softmax chains (ACT exp -> Pool select -> DVE sums) resolve
            def wo_closures(j, attnT_j):
                out = []
                for st in range(4):
                    for dp in range(2):
                        out.append(lambda st=st, dp=dp: wo_units(
                            j, attnT_j, st, [2 * dp, 2 * dp + 1]))
                return out

            qtrot = doubles.tile([128, H_LOC, SC], BF, tag="qtrot")
            projections0(qtrot)
            pending = None
            for j in range(nj):
                fillers = []
                punits = []
                qtrot_next = None
                if j + 1 < nj:
                    qtrot_next = doubles.tile([128, H_LOC, SC], BF,
                                              tag="qtrot")
                    punits = projection_units(j + 1, qtrot_next)
                wunits = wo_closures(j - 1, pending) if pending is not None \
                    else []
                # interleave so projection DVE bursts spread across heads
                for a, b in zip(wunits + [None] * len(punits),
                                punits + [None] * len(wunits)):
                    if a is not None:
                        fillers.append(a)
                    if b is not None:
                        fillers.append(b)
                pending = attention_and_wo(j, qtrot, fillers)
                qtrot = qtrot_next
            for st in range(4):
                wo_units(nj - 1, pending, st, range(4))

    nc.compile()
    return nc


def _host_inputs_causal(x, wq, wk, wv, wo, freqs_cos, freqs_sin):
    # half-layout column permutation within each head (even indices then odd)
    perm = np.concatenate([np.arange(0, 128, 2), np.arange(1, 128, 2)])

    def wproj_cols(w, g):
        cols = w[:, 512 * g:512 * (g + 1)].reshape(D, H_LOC, 128)
        return cols[:, :, perm].reshape(D, N_LOC)

    def as_dch(cols, dt):
        return np.ascontiguousarray(
            cols.reshape(DCH, 128, N_LOC).transpose(1, 0, 2)).astype(dt)

    def wqkb_arr(g):
        # [2(q|k), H_LOC, 128, DCH, 128] bf16, head-major for streaming
        out = np.empty((2, H_LOC, 128, DCH, 128), BF16)
        for wi, w in enumerate((wq, wk)):
            cols = wproj_cols(w, g).reshape(DCH, 128, H_LOC, 128)
            out[wi] = cols.transpose(2, 1, 0, 3).astype(BF16)
        return out

    def wv_arr(g, dt):
        cols = wv[:, 512 * g:512 * (g + 1)]
        return as_dch(cols, dt)

    def wo_arr(g, dt):
        rows = wo[512 * g:512 * (g + 1), :]
        return np.ascontiguousarray(
            rows.reshape(H_LOC, 128, D).transpose(1, 0, 2)).astype(dt)

    # cos/sin in half-layout: rows j and j+64 carry pair j's cos; sine rows
    # 0..63 = +sin (source a_j -> target j+64), rows 64..127 = -sin
    cosE = np.empty((128, S), np.float32)
    sinE = np.empty((128, S), np.float32)
    cosE[0:64] = freqs_cos.T
    cosE[64:128] = freqs_cos.T
    sinE[0:64] = freqs_sin.T
    sinE[64:128] = -freqs_sin.T

    xt0_b, xt8_b = [], []
    for b in range(B):
        xT = x[b].T  # [D, S] f32
        xt = xT.reshape(DCH, 128, NJ, SC).transpose(2, 1, 0, 3)
        xt0_b.append(np.ascontiguousarray(xt[0]).astype(BF16))
        xt8_b.append(np.ascontiguousarray(xt[1:]).astype(FP8NP))

    in_maps = []
    for c in range(N_CORES):
        b, g = c // 4, c % 4
        m = {
            "xt0": xt0_b[b], "xt8": xt8_b[b],
            "wqkb": wqkb_arr(g),
            "wvb": wv_arr(g, BF16), "wob": wo_arr(g, BF16),
            "wq8": as_dch(wproj_cols(wq, g), FP8NP),
            "wk8": as_dch(wproj_cols(wk, g), FP8NP),
            "wv8": wv_arr(g, FP8NP), "wo8": wo_arr(g, FP8NP),
            "cose": cosE, "sine": sinE,
        }
        in_maps.append(m)
    return in_maps


# ---------------------------------------------------------------------------
# legacy bf16 build for the non-causal variants (full attention / arbitrary
# additive mask) -- unchanged from the baseline implementation
# ---------------------------------------------------------------------------
def _build_legacy(variant: str, nj: int = NJ):
    use_mask = variant == "full_mask"

    nc = bacc.Bacc("TRN2", target_bir_lowering=False, debug=False,
                   num_devices=N_CORES)

    xt_d = nc.dram_tensor("xt", [NJ, 128, DCH, SC], BF, kind="ExternalInput").ap()
    wq_d = nc.dram_tensor("wq", [128, DCH, N_LOC], BF, kind="ExternalInput").ap()
    wk_d = nc.dram_tensor("wk", [128, DCH, N_LOC], BF, kind="ExternalInput").ap()
    wv_d = nc.dram_tensor("wv", [128, DCH, N_LOC], BF, kind="ExternalInput").ap()
    wo_d = nc.dram_tensor("wo", [128, H_LOC, D], BF, kind="ExternalInput").ap()
    cose_d = nc.dram_tensor("cose", [128, S], F32, kind="ExternalInput").ap()
    sine_d = nc.dram_tensor("sine", [128, S], F32, kind="ExternalInput").ap()
    maskt_d = None
    if use_mask:
        maskt_d = nc.dram_tensor("maskt", [NJ, 128, DCH, SC], BF,
                                 kind="ExternalInput").ap()
    out_d = nc.dram_tensor("out", [S, D], F32, kind="ExternalOutput").ap()
    out_v = out_d.rearrange("(a p) d -> a p d", p=128)

    with tile.TileContext(nc) as tc:
        with (
            tc.tile_pool(name="singles", bufs=1) as singles,
            tc.tile_pool(name="doubles", bufs=2) as doubles,
            tc.tile_pool(name="triples", bufs=3) as triples,
            tc.tile_pool(name="ps1", bufs=1, space="PSUM") as ps1,
            tc.tile_pool(name="ps2", bufs=2, space="PSUM") as ps2,
        ):
            rope_pool = doubles if use_mask else triples
            stage_pool = doubles if use_mask else triples
            epair_pool = doubles
            wq_sb = singles.tile([128, DCH, N_LOC], BF, tag="wq")
            wk_sb = singles.tile([128, DCH, N_LOC], BF, tag="wk")
            wv_sb = singles.tile([128, DCH, N_LOC], BF, tag="wv")
            wo_sb = singles.tile([128, H_LOC, D], BF, tag="wo")
            xt_pool = singles if use_mask else doubles
            xt0_sb = xt_pool.tile([128, DCH, SC], BF, tag="xt")
            for q4 in range(4):
                nc.sync.dma_start(out=wq_sb[:, 4 * q4:4 * (q4 + 1), :],
                                  in_=wq_d[:, 4 * q4:4 * (q4 + 1), :])
                nc.gpsimd.dma_start(out=xt0_sb[:, 4 * q4:4 * (q4 + 1), :],
                                    in_=xt_d[0][:, 4 * q4:4 * (q4 + 1), :])
            cose_sb = singles.tile([128, S], F32, tag="cose")
            sine_sb = singles.tile([128, S], F32, tag="sine")
            nc.sync.dma_start(out=cose_sb[:], in_=cose_d[:])
            nc.sync.dma_start(out=sine_sb[:], in_=sine_d[:])
            nc.sync.dma_start(out=wk_sb[:], in_=wk_d[:])
            nc.sync.dma_start(out=wv_sb[:], in_=wv_d[:])
            nc.sync.dma_start(out=wo_sb[:], in_=wo_d[:])
            from concourse.masks import make_identity
            ones_sb = singles.tile([128, 128], BF, tag="ones")
            nc.vector.memset(ones_sb[:], 1.0)
            ident_sb = singles.tile([128, 128], BF, tag="ident")
            make_identity(nc, ident_sb[:])
            ktrot = singles.tile([128, H_LOC, S], BF, tag="ktrot")
            v_sb = singles.tile([128, NJ * H_LOC, SC], BF, tag="v")
            qtrot_all = singles.tile([128, H_LOC, S], BF, tag="qtrot_all")

            def projections(j, qdest, qsl, xt_pre=None):
                js = j * SC
                if xt_pre is not None:
                    xt_sb = xt_pre
                else:
                    xt_sb = xt_pool.tile([128, DCH, SC], BF, tag="xt")
                    nc.gpsimd.dma_start(out=xt_sb[:], in_=xt_d[j])

                for w_sb, dest, dsl in ((wq_sb, qdest, qsl),
                                        (wk_sb, ktrot, slice(js, js + SC))):
                    for hp in range(2):
                        parts = []
                        for hh in range(2):
                            h = hp * 2 + hh
                            if j == 0 and hp == 1:
                                ps = ps1.tile([128, SC], F32,
                                              tag="pv" if hh else "sums")
                            else:
                                ps = ps2.tile([128, SC], F32, tag="qkv1")
                            for d in range(DCH):
                                nc.tensor.matmul(
                                    ps[:],
                                    w_sb[:, d, h * 128:(h + 1) * 128],
                                    xt_sb[:, d, :],
                                    start=(d == 0), stop=(d == DCH - 1),
                                )
                            a_sb = rope_pool.tile([128, SC], F32, tag="ropeA")
                            nc.vector.tensor_mul(
                                a_sb[:], ps[:], cose_sb[:, js:js + SC])
                            b_sb = triples.tile([128, SC], F32, tag="ropeB")
                            nc.vector.tensor_mul(
                                b_sb[:], ps[:], sine_sb[:, js:js + SC])
                            b2_sb = triples.tile([128, SC], F32, tag="ropeB2")
                            nc.scalar.dma_start(out=b2_sb[0:64, :],
                                                in_=b_sb[64:128, :])
                            nc.scalar.dma_start(out=b2_sb[64:128, :],
                                                in_=b_sb[0:64, :])
                            parts.append((h, a_sb, b2_sb))
                        for h, a_sb, b2_sb in parts:
                            if dsl is None:
                                dst = dest[:, h, :]
                            else:
                                dst = dest[:, h, dsl]
                            nc.vector.tensor_add(dst, a_sb[:], b2_sb[:])

                for tl in range(4):
                    ps = ps2.tile([128, SC], F32, tag="qkv1")
                    for d in range(DCH):
                        nc.tensor.matmul(
                            ps[:],
                            xt_sb[:, d, tl * 128:(tl + 1) * 128],
                            wv_sb[:, d, :],
                            start=(d == 0), stop=(d == DCH - 1),
                        )
                    nc.scalar.copy(out=v_sb[:, 4 * j + tl, :], in_=ps[:])

            def wo_units(j, attnT_j, st, dcs):
                for dc in dcs:
                    wps = ps2.tile([128, SC], F32, tag="qkv1")
                    for h2 in range(H_LOC):
                        nc.tensor.matmul(
                            wps[:],
                            attnT_j[:, h2, st * 128:(st + 1) * 128],
                            wo_sb[:, h2, dc * SC:(dc + 1) * SC],
                            start=(h2 == 0), stop=(h2 == H_LOC - 1),
                        )
                    o_sb = stage_pool.tile([128, SC], F32, tag="ostage")
                    if (st + dc) % 2 == 0:
                        nc.scalar.copy(out=o_sb[:], in_=wps[:])
                    else:
                        nc.vector.tensor_copy(o_sb[:], wps[:])
                    nc.sync.dma_start(
                        out=out_v[4 * j + st][:, dc * SC:(dc + 1) * SC],
                        in_=o_sb[:])

            def attention_and_wo(j, qtrot_h, prev=None):
                maskt_sb = None
                if use_mask:
                    maskt_sb = xt_pool.tile([128, DCH, SC], BF, tag="xt")
                    nc.sync.dma_start(out=maskt_sb[:], in_=maskt_d[j])

                attnT_j = doubles.tile([128, H_LOC, SC], BF, tag="attnT")
                pg_order = list(range(DCH // 2))
                for h in range(H_LOC):
                    sums_ps = ps1.tile([128, SC], F32, tag="sums")
                    pv_ps = ps1.tile([128, SC], F32, tag="pv")
                    for gi, pg in enumerate(pg_order):
                        sc_ps = ps2.tile([128, 2, SC], F32, tag="sc")
                        exp_sb = stage_pool.tile([128, 2, SC], BF, tag="exp")
                        for i_ in range(2):
                            tt = pg * 2 + i_
                            nc.tensor.matmul(
                                sc_ps[:, i_, :],
                                ktrot[:, h, tt * 128:(tt + 1) * 128],
                                qtrot_h(h),
                                start=True, stop=not use_mask,
                            )
                            if use_mask:
                                nc.tensor.matmul(
                                    sc_ps[:, i_, :], ident_sb[:],
                                    maskt_sb[:, tt, :],
                                    start=False, stop=True,
                                )
                        nc.scalar.activation(out=exp_sb[:], in_=sc_ps[:],
                                             func=AF.Exp, scale=SCALE)
                        epair = epair_pool.tile([128, SC], BF, tag="epair")
                        nc.vector.tensor_add(epair[:], exp_sb[:, 0, :],
                                             exp_sb[:, 1, :])
                        nc.tensor.matmul(sums_ps[:], ones_sb[:], epair[:],
                                         start=gi == 0,
                                         stop=gi == len(pg_order) - 1)
                        for i_ in range(2):
                            tt = pg * 2 + i_
                            first = gi == 0 and i_ == 0
                            last = gi == len(pg_order) - 1 and i_ == 1
                            nc.tensor.matmul(pv_ps[:],
                                             v_sb[:, tt, h * 128:(h + 1) * 128],
                                             exp_sb[:, i_, :],
                                             start=first, stop=last)
                    recip_sb = doubles.tile([128, SC], F32, tag="recip")
                    nc.vector.reciprocal_approx_fast(out=recip_sb[:], in_=sums_ps[:])
                    nc.vector.tensor_mul(attnT_j[:, h, :], pv_ps[:], recip_sb[:])
                    if prev is not None:
                        wo_units(j - 1, prev, h, range(4))
                return attnT_j

            pending = None
            for j in range(nj):
                projections(j, qtrot_all, slice(j * SC, (j + 1) * SC),
                            xt_pre=xt0_sb if j == 0 else None)
            for j in range(nj):
                js = j * SC
                pending = attention_and_wo(
                    j, lambda h, js=js: qtrot_all[:, h, js:js + SC],
                    prev=pending)
            for st in range(4):
                wo_units(nj - 1, pending, st, range(4))

    nc.compile()
    return nc


def _get_build(variant):
    if variant not in _BUILDS:
        if variant == "causal":
            _BUILDS[variant] = _build_causal()
        else:
            _BUILDS[variant] = _build_legacy(variant)
    return _BUILDS[variant]


def _classify_mask(mask):
    if not np.any(mask):
        return "full_nomask"
    tril = np.tril(np.ones((S, S), dtype=bool))
    if np.all(mask[tril] == 0.0) and np.all(mask[~tril] <= -1e9):
        return "causal"
    return "full_mask"


def kernel(x, wq, wk, wv, wo, freqs_cos, freqs_sin, mask):
    global LAST_RESULT
    x = np.asarray(x, dtype=np.float32)
    wq, wk, wv, wo = (np.asarray(w, dtype=np.float32)
                      for w in (wq, wk, wv, wo))
    freqs_cos = np.asarray(freqs_cos, dtype=np.float32)
    freqs_sin = np.asarray(freqs_sin, dtype=np.float32)
    mask = np.asarray(mask, dtype=np.float32)

    variant = _classify_mask(mask)
    nc = _get_build(variant)

    if variant == "causal":
        in_maps = _host_inputs_causal(x, wq, wk, wv, wo, freqs_cos, freqs_sin)
    else:
        in_maps = _host_inputs_legacy(x, wq, wk, wv, wo, freqs_cos,
                                      freqs_sin, mask, variant)

    res = run_bass_kernel_spmd(nc, in_maps, list(range(N_CORES)))
    LAST_RESULT = res
    outs = [res.results[c]["out"].astype(np.float32) for c in range(N_CORES)]
    out = np.stack([
        outs[0] + outs[1] + outs[2] + outs[3],
        outs[4] + outs[5] + outs[6] + outs[7],
    ]).astype(np.float32)
    return out


def _host_inputs_legacy(x, wq, wk, wv, wo, freqs_cos, freqs_sin, mask,
                        variant):
    perm = np.concatenate([np.arange(0, 128, 2), np.arange(1, 128, 2)])

    def wproj_arr(w, g):
        cols = w[:, 512 * g:512 * (g + 1)].reshape(D, H_LOC, 128)
        cols = cols[:, :, perm].reshape(D, N_LOC)
        return np.ascontiguousarray(
            cols.reshape(DCH, 128, N_LOC).transpose(1, 0, 2)).astype(BF16)

    def wv_arr(w, g):
        cols = w[:, 512 * g:512 * (g + 1)]
        return np.ascontiguousarray(
            cols.reshape(DCH, 128, N_LOC).transpose(1, 0, 2)).astype(BF16)

    def wo_arr(g):
        rows = wo[512 * g:512 * (g + 1), :]
        return np.ascontiguousarray(
            rows.reshape(H_LOC, 128, D).transpose(1, 0, 2)).astype(BF16)

    cosE = np.empty((128, S), np.float32)
    sinE = np.empty((128, S), np.float32)
    cosE[0:64] = freqs_cos.T
    cosE[64:128] = freqs_cos.T
    sinE[0:64] = freqs_sin.T
    sinE[64:128] = -freqs_sin.T

    xt_b = []
    for b in range(B):
        xT = x[b].T.astype(BF16)
        xt = np.ascontiguousarray(
            xT.reshape(DCH, 128, NJ, SC).transpose(2, 1, 0, 3))
        xt_b.append(xt)

    maskt = None
    if variant == "full_mask":
        mT = (mask.T / SCALE).astype(BF16)
        maskt = np.ascontiguousarray(
            mT.reshape(DCH, 128, NJ, SC).transpose(2, 1, 0, 3))

    wq_g = [wproj_arr(wq, g) for g in range(H_LOC)]
    wk_g = [wproj_arr(wk, g) for g in range(H_LOC)]
    wv_g = [wv_arr(wv, g) for g in range(H_LOC)]
    wo_g = [wo_arr(g) for g in range(H_LOC)]

    in_maps = []
    for c in range(N_CORES):
        b, g = c // 4, c % 4
        m = {
            "xt": xt_b[b],
            "wq": wq_g[g], "wk": wk_g[g], "wv": wv_g[g], "wo": wo_g[g],
            "cose": cosE, "sine": sinE,
        }
        if maskt is not None:
            m["maskt"] = maskt
        in_maps.append(m)
    return in_maps


# revision 45
# speedup vs baseline: 1.1389x; 1.1389x over previous
"""Multi-head causal attention with RoPE on 8 Trainium2 NeuronCores.

Sharding: data-parallel over batch (2) x tensor-parallel over heads (16 -> 4
per core). Each core computes q/k/v projections for its 4 heads on its batch
element, attention, and a partial output projection (its rows of wo); the
host sums the 4 partials per batch element.

Device-side layout: everything is computed "transposed" (scores held as
[t, s]) so no on-device transposes are needed anywhere; softmax denominators
come from an all-ones matmul (partition reduction on the tensor engine).
RoPE pair-swap is done by permuting the wq/wk columns on the host into
(even|odd) half-layout so the swap becomes two partition-halved SBUF->SBUF
DMA copies.

Speed scheme (causal variant): hybrid precision. The first sequence chunk
(queries s<512 / keys t<512, where causal attention is concentrated and
quantization errors are amplified) runs in bf16; chunks 1-3 run their
q/k/v projections, attn@v, and output projection as fp8e4m3 DoubleRow
matmuls (2x tensor-engine throughput; two 128-deep K-tiles per pass).
Scores and softmax-denominator matmuls stay bf16 everywhere. The causal
diagonal mask is applied by zeroing exp tiles with affine_select on the
(otherwise idle) Pool engine instead of identity-matmul mask adds on the
PE. exp uses a -1 bias shift so fp8 exp tiles cannot overflow (max logit
~6.0 -> e^5.0 = 148 < 240); the shift cancels in the softmax ratio.
"""

import math

import ml_dtypes
import numpy as np

import concourse.bass as bass
import concourse.mybir as mybir
import concourse.tile as tile
from concourse import bacc
from concourse.bass_utils import run_bass_kernel_spmd

BF16 = ml_dtypes.bfloat16
FP8NP = ml_dtypes.float8_e4m3
F32 = mybir.dt.float32
BF = mybir.dt.bfloat16
FP8 = mybir.dt.float8e4
AF = mybir.ActivationFunctionType
DRMODE = mybir.MatmulPerfMode.DoubleRow

N_CORES = 8
B = 2
S = 2048
D = 2048
H = 16
HD = 128
H_LOC = 4          # heads per core
N_LOC = H_LOC * HD  # 512 local head dims
NJ = 4             # s-chunks
SC = S // NJ       # 512 s-chunk width
DCH = D // 128     # 16 contraction chunks
SCALE = 1.0 / math.sqrt(HD)
EXP_BIAS = -1.0    # exp(SCALE*score - 1): keeps fp8 exp tiles under 240

_BUILDS: dict = {}
LAST_RESULT = None


def _build_causal(nj: int = NJ):
    nc = bacc.Bacc("TRN2", target_bir_lowering=False, debug=False,
                   num_devices=N_CORES)

    # chunk 0 inputs (bf16)
    xt0_d = nc.dram_tensor("xt0", [128, DCH, SC], BF, kind="ExternalInput").ap()
    wqkb_d = nc.dram_tensor("wqkb", [2, H_LOC, 128, DCH, 128], BF,
                            kind="ExternalInput").ap()  # head-major q|k bf16
    wvb_d = nc.dram_tensor("wvb", [128, DCH, N_LOC], BF, kind="ExternalInput").ap()
    wob_d = nc.dram_tensor("wob", [128, H_LOC, D], BF, kind="ExternalInput").ap()
    # chunks 1-3 inputs (fp8)
    xt8_d = nc.dram_tensor("xt8", [NJ - 1, 128, DCH, SC], FP8,
                           kind="ExternalInput").ap()
    wq8_d = nc.dram_tensor("wq8", [128, DCH, N_LOC], FP8, kind="ExternalInput").ap()
    wk8_d = nc.dram_tensor("wk8", [128, DCH, N_LOC], FP8, kind="ExternalInput").ap()
    wv8_d = nc.dram_tensor("wv8", [128, DCH, N_LOC], FP8, kind="ExternalInput").ap()
    wo8_d = nc.dram_tensor("wo8", [128, H_LOC, D], FP8, kind="ExternalInput").ap()
    cose_d = nc.dram_tensor("cose", [128, S], F32, kind="ExternalInput").ap()
    sine_d = nc.dram_tensor("sine", [128, S], F32, kind="ExternalInput").ap()
    out_d = nc.dram_tensor("out", [S, D], BF, kind="ExternalOutput").ap()
    out_v = out_d.rearrange("(a p) d -> a p d", p=128)

    with tile.TileContext(nc) as tc:
        with (
            tc.tile_pool(name="singles", bufs=1) as singles,
            tc.tile_pool(name="doubles", bufs=2) as doubles,
            tc.tile_pool(name="triples", bufs=3) as triples,
            tc.tile_pool(name="quads", bufs=4) as quads,
            tc.tile_pool(name="hexes", bufs=7) as hexes,
            tc.tile_pool(name="ps1", bufs=1, space="PSUM") as ps1,
            tc.tile_pool(name="ps2", bufs=2, space="PSUM") as ps2,
        ):
            # ---- persistent tensors ----
            # chunk-0 q/k weights are streamed per head from wqkb_d; wv/wo
            # bf16 stay resident (v uses full-width rhs; wo reused per dc).
            wvb_sb = singles.tile([128, DCH, N_LOC], BF, tag="wvb")
            wob_sb = singles.tile([128, H_LOC, D], BF, tag="wob")
            wq8_sb = singles.tile([128, DCH, N_LOC], FP8, tag="wq8")
            wk8_sb = singles.tile([128, DCH, N_LOC], FP8, tag="wk8")
            wv8_sb = singles.tile([128, DCH, N_LOC], FP8, tag="wv8")
            wo8_sb = singles.tile([128, H_LOC, D], FP8, tag="wo8")
            xt0_sb = singles.tile([128, DCH, SC], BF, tag="xt0")
            # startup DMA order: xt0 pieces on the gpsimd queue, q/k weight
            # streams (piecewise) on sync in consumption order, cos/sin on
            # the scalar queue, later-needed tensors behind.
            for q4 in range(4):
                nc.gpsimd.dma_start(out=xt0_sb[:, 4 * q4:4 * (q4 + 1), :],
                                    in_=xt0_d[:, 4 * q4:4 * (q4 + 1), :])
            wqk_sb = {}
            for wi in range(2):
                for h in range(H_LOC):
                    wt_tile = triples.tile([128, DCH, 128], BF, tag="wstream")
                    for piece in range(2):
                        nc.sync.dma_start(
                            out=wt_tile[:, 8 * piece:8 * (piece + 1), :],
                            in_=wqkb_d[wi, h][:, 8 * piece:8 * (piece + 1), :])
                    wqk_sb[(wi, h)] = wt_tile
            cose_sb = singles.tile([128, S], F32, tag="cose")
            sine_sb = singles.tile([128, S], F32, tag="sine")
            nc.scalar.dma_start(out=cose_sb[:], in_=cose_d[:])
            nc.scalar.dma_start(out=sine_sb[:], in_=sine_d[:])
            nc.sync.dma_start(out=wvb_sb[:], in_=wvb_d[:])
            nc.sync.dma_start(out=wq8_sb[:], in_=wq8_d[:])
            nc.sync.dma_start(out=wk8_sb[:], in_=wk8_d[:])
            nc.sync.dma_start(out=wv8_sb[:], in_=wv8_d[:])
            nc.sync.dma_start(out=wob_sb[:], in_=wob_d[:])
            nc.sync.dma_start(out=wo8_sb[:], in_=wo8_d[:])
            ones_sb = singles.tile([128, 128], BF, tag="ones")
            nc.vector.memset(ones_sb[:], 1.0)
            bias_sb = singles.tile([128, 1], F32, tag="bias")
            nc.vector.memset(bias_sb[:], EXP_BIAS)
            # k^T (rotated, bf16) accumulates across chunks; v in fp8 for
            # DR pv (all chunks) + bf16 copy of t-chunk 0 for chunk-0 pv
            ktrot = singles.tile([128, H_LOC, S], BF, tag="ktrot")
            v8_sb = singles.tile([128, NJ * H_LOC, SC], FP8, tag="v8")
            vbf_sb = singles.tile([128, H_LOC, SC], BF, tag="vbf")

            def qk_unit(j, xt_sb, wi, hp, qdest):
                """DR/bf16 projections + rope for heads hp*2, hp*2+1 of
                q (wi=0) or k (wi=1) of chunk j."""
                js = j * SC
                dest, dsl = ((qdest, None) if wi == 0 else
                             (ktrot, slice(js, js + SC)))
                w8_sb = wq8_sb if wi == 0 else wk8_sb
                parts = []
                for hh in range(2):
                    h = hp * 2 + hh
                    if j == 0 and hp == 1:
                        ps = ps1.tile([128, SC], F32,
                                      tag="pv" if hh else "sums")
                    else:
                        ps = ps2.tile([128, SC], F32, tag="qkv1")
                    if j == 0:
                        wt = wqk_sb[(wi, h)]
                        for d in range(DCH):
                            nc.tensor.matmul(
                                ps[:], wt[:, d, :], xt_sb[:, d, :],
                                start=(d == 0), stop=(d == DCH - 1),
                            )
                    else:
                        for d in range(DCH // 2):
                            nc.tensor.matmul(
                                ps[:],
                                w8_sb[:, 2 * d:2 * d + 2,
                                      h * 128:(h + 1) * 128],
                                xt_sb[:, 2 * d:2 * d + 2, :],
                                start=(d == 0),
                                stop=(d == DCH // 2 - 1),
                                perf_mode=DRMODE,
                            )
                    a_sb = quads.tile([128, SC], F32, tag="ropeA")
                    nc.vector.tensor_mul(
                        a_sb[:], ps[:], cose_sb[:, js:js + SC])
                    b_sb = triples.tile([128, SC], F32, tag="ropeB")
                    nc.vector.tensor_mul(
                        b_sb[:], ps[:], sine_sb[:, js:js + SC])
                    # half-swap via SBUF->SBUF DMA (cross-partition)
                    b2_sb = triples.tile([128, SC], F32, tag="ropeB2")
                    nc.scalar.dma_start(out=b2_sb[0:64, :],
                                        in_=b_sb[64:128, :])
                    nc.scalar.dma_start(out=b2_sb[64:128, :],
                                        in_=b_sb[0:64, :])
                    parts.append((h, a_sb, b2_sb))
                for h, a_sb, b2_sb in parts:
                    if dsl is None:
                        dst = dest[:, h, :]
                    else:
                        dst = dest[:, h, dsl]
                    nc.vector.tensor_add(dst, a_sb[:], b2_sb[:])

            def v_unit(j, xt_sb, tl):
                ps = ps2.tile([128, SC], F32, tag="qkv1")
                if j == 0:
                    for d in range(DCH):
                        nc.tensor.matmul(
                            ps[:],
                            xt_sb[:, d, tl * 128:(tl + 1) * 128],
                            wvb_sb[:, d, :],
                            start=(d == 0), stop=(d == DCH - 1),
                        )
                    nc.scalar.copy(out=vbf_sb[:, tl, :], in_=ps[:])
                    # pool can't read PSUM; mirror to fp8 from the SBUF copy
                    nc.gpsimd.tensor_copy(v8_sb[:, tl, :], vbf_sb[:, tl, :])
                else:
                    for d in range(DCH // 2):
                        nc.tensor.matmul(
                            ps[:],
                            xt_sb[:, 2 * d:2 * d + 2,
                                  tl * 128:(tl + 1) * 128],
                            wv8_sb[:, 2 * d:2 * d + 2, :],
                            start=(d == 0), stop=(d == DCH // 2 - 1),
                            perf_mode=DRMODE,
                        )
                    nc.scalar.copy(out=v8_sb[:, 4 * j + tl, :], in_=ps[:])

            def projection_units(j, qdest):
                """Emit the xt DMA now; return per-unit closures for the
                matmul/rope work (interleaved into the previous chunk's
                attention as PE bubble fillers)."""
                # sync queue (idle after startup): the gpsimd queue would
                # serialize these triggers behind pool-engine compute
                xt_sb = doubles.tile([128, DCH, SC], FP8, tag="xt8")
                for q4 in range(4):
                    nc.sync.dma_start(
                        out=xt_sb[:, 4 * q4:4 * (q4 + 1), :],
                        in_=xt8_d[j - 1][:, 4 * q4:4 * (q4 + 1), :])
                units = []
                for wi in range(2):
                    for hp in range(2):
                        units.append(lambda wi=wi, hp=hp:
                                     qk_unit(j, xt_sb, wi, hp, qdest))
                for tl in range(4):
                    units.append(lambda tl=tl: v_unit(j, xt_sb, tl))
                return units

            def projections0(qdest):
                for wi in range(2):
                    for hp in range(2):
                        qk_unit(0, xt0_sb, wi, hp, qdest)
                for tl in range(4):
                    v_unit(0, xt0_sb, tl)

            def wo_units(j, attnT_j, st, dcs):
                for dc in dcs:
                    wps = ps2.tile([128, SC], F32, tag="qkv1")
                    if j == 0:
                        for h2 in range(H_LOC):
                            nc.tensor.matmul(
                                wps[:],
                                attnT_j[:, h2, st * 128:(st + 1) * 128],
                                wob_sb[:, h2, dc * SC:(dc + 1) * SC],
                                start=(h2 == 0), stop=(h2 == H_LOC - 1),
                            )
                    else:
                        for hp in range(H_LOC // 2):
                            nc.tensor.matmul(
                                wps[:],
                                attnT_j[:, 2 * hp:2 * hp + 2,
                                        st * 128:(st + 1) * 128],
                                wo8_sb[:, 2 * hp:2 * hp + 2,
                                       dc * SC:(dc + 1) * SC],
                                start=(hp == 0), stop=(hp == H_LOC // 2 - 1),
                                perf_mode=DRMODE,
                            )
                    o_sb = triples.tile([128, SC], BF, tag="ostage")
                    if (st + dc) % 2 == 0:
                        nc.scalar.copy(out=o_sb[:], in_=wps[:])
                    else:
                        nc.vector.tensor_copy(o_sb[:], wps[:])
                    nc.sync.dma_start(
                        out=out_v[4 * j + st][:, dc * SC:(dc + 1) * SC],
                        in_=o_sb[:])

            def attention_and_wo(j, qtrot, fillers=(), self_wo=False):
                fillers = list(fillers)
                emitted = [0]

                def pump(frac):
                    # keep the in-order PE queue fed: emit filler units up to
                    # the given fraction of attention progress
                    n_emit = min(len(fillers),
                                 int(len(fillers) * frac + 0.999))
                    while emitted[0] < n_emit:
                        fillers[emitted[0]]()
                        emitted[0] += 1

                bf = j == 0
                edt = BF if bf else FP8
                if bf:
                    attnT_j = singles.tile([128, H_LOC, SC], BF, tag="attnT_bf")
                else:
                    attnT_j = doubles.tile([128, H_LOC, SC], FP8, tag="attnT8")
                # diagonal pair-groups first: their longer select+exp chain
                # overlaps the remaining full groups' matmuls
                pg_order = list(range(2 * j, 2 * j + 2)) + list(range(2 * j))
                ng = len(pg_order)
                for h in range(H_LOC):
                    sums_ps = ps1.tile([128, SC], F32, tag="sums")
                    pv_ps = ps1.tile([128, SC], F32, tag="pv")

                    def emit_scores_exp(gi):
                        """scores + exp/select/epair chain for group gi;
                        returns (exp_sb, equad-or-None)."""
                        pg = pg_order[gi]
                        sc_ps = ps2.tile([128, 2, SC], F32, tag="sc")
                        if bf:
                            exp_sb = doubles.tile([128, 2, SC], BF,
                                                  tag="exp_bf")
                        else:
                            exp_sb = triples.tile([128, 2, SC], FP8, tag="exp8")
                        diag = pg >= 2 * j
                        for i_ in range(2):
                            tt = pg * 2 + i_
                            nc.tensor.matmul(
                                sc_ps[:, i_, :],
                                ktrot[:, h, tt * 128:(tt + 1) * 128],
                                qtrot[:, h, :],
                                start=True, stop=True,
                            )
                        if diag:
                            # per-block exp so the Pool select of block 0 can
                            # start while block 1's exp is still on ACT
                            for i_ in range(2):
                                nc.scalar.activation(
                                    out=exp_sb[:, i_, :], in_=sc_ps[:, i_, :],
                                    func=AF.Exp, scale=SCALE, bias=bias_sb[:])
                                p = pg * 2 + i_ - 4 * j
                                nc.gpsimd.affine_select(
                                    exp_sb[:, i_, :], exp_sb[:, i_, :],
                                    pattern=[[1, SC]],
                                    compare_op=mybir.AluOpType.is_ge,
                                    fill=0.0, base=-128 * p,
                                    channel_multiplier=-1)
                        else:
                            nc.scalar.activation(out=exp_sb[:], in_=sc_ps[:],
                                                 func=AF.Exp, scale=SCALE,
                                                 bias=bias_sb[:])
                        epair = doubles.tile([128, SC], BF, tag="epair")
                        nc.vector.tensor_add(epair[:], exp_sb[:, 0, :],
                                             exp_sb[:, 1, :])
                        equad = None
                        if gi % 2 == 0:
                            pend_epair[0] = epair
                        else:
                            equad = triples.tile([128, SC], BF, tag="equad")
                            nc.vector.tensor_add(equad[:], pend_epair[0][:],
                                                 epair[:])
                        return exp_sb, equad

                    def emit_pe_consumers(gi, exp_sb, equad):
                        pg = pg_order[gi]
                        if equad is not None:
                            nc.tensor.matmul(sums_ps[:], ones_sb[:], equad[:],
                                             start=gi == 1, stop=gi == ng - 1)
                        if bf:
                            for i_ in range(2):
                                tt = pg * 2 + i_
                                nc.tensor.matmul(
                                    pv_ps[:],
                                    vbf_sb[:, tt, h * 128:(h + 1) * 128],
                                    exp_sb[:, i_, :],
                                    start=gi == 0 and i_ == 0,
                                    stop=gi == ng - 1 and i_ == 1)
                        else:
                            nc.tensor.matmul(
                                pv_ps[:],
                                v8_sb[:, 2 * pg:2 * pg + 2,
                                      h * 128:(h + 1) * 128],
                                exp_sb[:, :, :],
                                start=gi == 0, stop=gi == ng - 1,
                                perf_mode=DRMODE)

                    # depth-1 software pipeline: group gi's scores run on the
                    # PE while group gi-1's exp/select chain completes, so the
                    # ones/pv consumers of gi-1 issue with their deps resolved
                    pend_epair = [None]
                    pending_grp = None
                    for gi in range(ng):
                        made = emit_scores_exp(gi)
                        if pending_grp is not None:
                            emit_pe_consumers(*pending_grp)
                        pending_grp = (gi, *made)
                    emit_pe_consumers(*pending_grp)
                    recip_sb = doubles.tile([128, SC], F32, tag="recip")
                    nc.vector.reciprocal_approx_fast(out=recip_sb[:],
                                                     in_=sums_ps[:])
                    nc.vector.tensor_mul(attnT_j[:, h, :], pv_ps[:],
                                         recip_sb[:])
                    pump((h + 1) / H_LOC if h < H_LOC - 1 else 1.0)
                return attnT_j

            # prev-chunk wo and next-chunk projections run interleaved with
            # each chunk's attention, keeping the in-order PE queue fed while
            # softmax chains (ACT exp -> Pool select -> DVE sums) resolve
            def wo_closures(j, attnT_j):
                out = []
                for st in range(4):
                    for dp in range(2):
                        out.append(lambda st=st, dp=dp: wo_units(
                            j, attnT_j, st, [2 * dp, 2 * dp + 1]))
                return out

            qtrot = doubles.tile([128, H_LOC, SC], BF, tag="qtrot")
            projections0(qtrot)
            pending = None
            for j in range(nj):
                fillers = []
                punits = []
                qtrot_next = None
                if j + 1 < nj:
                    qtrot_next = doubles.tile([128, H_LOC, SC], BF,
                                              tag="qtrot")
                    punits = projection_units(j + 1, qtrot_next)
                wunits = wo_closures(j - 1, pending) if pending is not None \
                    else []
                # interleave so projection DVE bursts spread across heads
                for a, b in zip(wunits + [None] * len(punits),
                                punits + [None] * len(wunits)):
                    if a is not None:
                        fillers.append(a)
                    if b is not None:
                        fillers.append(b)
                pending = attention_and_wo(j, qtrot, fillers)
                qtrot = qtrot_next
            for st in range(4):
                wo_units(nj - 1, pending, st, range(4))

    nc.compile()
    return nc


def _host_inputs_causal(x, wq, wk, wv, wo, freqs_cos, freqs_sin):
    # half-layout column permutation within each head (even indices then odd)
    perm = np.concatenate([np.arange(0, 128, 2), np.arange(1, 128, 2)])

    def wproj_cols(w, g):
        cols = w[:, 512 * g:512 * (g + 1)].reshape(D, H_LOC, 128)
        return cols[:, :, perm].reshape(D, N_LOC)

    def as_dch(cols, dt):
        return np.ascontiguousarray(
            cols.reshape(DCH, 128, N_LOC).transpose(1, 0, 2)).astype(dt)

    def wqkb_arr(g):
        # [2(q|k), H_LOC, 128, DCH, 128] bf16, head-major for streaming
        out = np.empty((2, H_LOC, 128, DCH, 128), BF16)
        for wi, w in enumerate((wq, wk)):
            cols = wproj_cols(w, g).reshape(DCH, 128, H_LOC, 128)
            out[wi] = cols.transpose(2, 1, 0, 3).astype(BF16)
        return out

    def wv_arr(g, dt):
        cols = wv[:, 512 * g:512 * (g + 1)]
        return as_dch(cols, dt)

    def wo_arr(g, dt):
        rows = wo[512 * g:512 * (g + 1), :]
        return np.ascontiguousarray(
            rows.reshape(H_LOC, 128, D).transpose(1, 0, 2)).astype(dt)

    # cos/sin in half-layout: rows j and j+64 carry pair j's cos; sine rows
    # 0..63 = +sin (source a_j -> target j+64), rows 64..127 = -sin
    cosE = np.empty((128, S), np.float32)
    sinE = np.empty((128, S), np.float32)
    cosE[0:64] = freqs_cos.T
    cosE[64:128] = freqs_cos.T
    sinE[0:64] = freqs_sin.T
    sinE[64:128] = -freqs_sin.T

    xt0_b, xt8_b = [], []
    for b in range(B):
        xT = x[b].T  # [D, S] f32
        xt = xT.reshape(DCH, 128, NJ, SC).transpose(2, 1, 0, 3)
        xt0_b.append(np.ascontiguousarray(xt[0]).astype(BF16))
        xt8_b.append(np.ascontiguousarray(xt[1:]).astype(FP8NP))

    in_maps = []
    for c in range(N_CORES):
        b, g = c // 4, c % 4
        m = {
            "xt0": xt0_b[b], "xt8": xt8_b[b],
            "wqkb": wqkb_arr(g),
            "wvb": wv_arr(g, BF16), "wob": wo_arr(g, BF16),
            "wq8": as_dch(wproj_cols(wq, g), FP8NP),
            "wk8": as_dch(wproj_cols(wk, g), FP8NP),
            "wv8": wv_arr(g, FP8NP), "wo8": wo_arr(g, FP8NP),
            "cose": cosE, "sine": sinE,
        }
        in_maps.append(m)
    return in_maps


# ---------------------------------------------------------------------------
# legacy bf16 build for the non-causal variants (full attention / arbitrary
# additive mask) -- unchanged from the baseline implementation
# ---------------------------------------------------------------------------
def _build_legacy(variant: str, nj: int = NJ):
    use_mask = variant == "full_mask"

    nc = bacc.Bacc("TRN2", target_bir_lowering=False, debug=False,
                   num_devices=N_CORES)

    xt_d = nc.dram_tensor("xt", [NJ, 128, DCH, SC], BF, kind="ExternalInput").ap()
    wq_d = nc.dram_tensor("wq", [128, DCH, N_LOC], BF, kind="ExternalInput").ap()
    wk_d = nc.dram_tensor("wk", [128, DCH, N_LOC], BF, kind="ExternalInput").ap()
    wv_d = nc.dram_tensor("wv", [128, DCH, N_LOC], BF, kind="ExternalInput").ap()
    wo_d = nc.dram_tensor("wo", [128, H_LOC, D], BF, kind="ExternalInput").ap()
    cose_d = nc.dram_tensor("cose", [128, S], F32, kind="ExternalInput").ap()
    sine_d = nc.dram_tensor("sine", [128, S], F32, kind="ExternalInput").ap()
    maskt_d = None
    if use_mask:
        maskt_d = nc.dram_tensor("maskt", [NJ, 128, DCH, SC], BF,
                                 kind="ExternalInput").ap()
    out_d = nc.dram_tensor("out", [S, D], F32, kind="ExternalOutput").ap()
    out_v = out_d.rearrange("(a p) d -> a p d", p=128)

    with tile.TileContext(nc) as tc:
        with (
            tc.tile_pool(name="singles", bufs=1) as singles,
            tc.tile_pool(name="doubles", bufs=2) as doubles,
            tc.tile_pool(name="triples", bufs=3) as triples,
            tc.tile_pool(name="ps1", bufs=1, space="PSUM") as ps1,
            tc.tile_pool(name="ps2", bufs=2, space="PSUM") as ps2,
        ):
            rope_pool = doubles if use_mask else triples
            stage_pool = doubles if use_mask else triples
            epair_pool = doubles
            wq_sb = singles.tile([128, DCH, N_LOC], BF, tag="wq")
            wk_sb = singles.tile([128, DCH, N_LOC], BF, tag="wk")
            wv_sb = singles.tile([128, DCH, N_LOC], BF, tag="wv")
            wo_sb = singles.tile([128, H_LOC, D], BF, tag="wo")
            xt_pool = singles if use_mask else doubles
            xt0_sb = xt_pool.tile([128, DCH, SC], BF, tag="xt")
            for q4 in range(4):
                nc.sync.dma_start(out=wq_sb[:, 4 * q4:4 * (q4 + 1), :],
                                  in_=wq_d[:, 4 * q4:4 * (q4 + 1), :])
                nc.gpsimd.dma_start(out=xt0_sb[:, 4 * q4:4 * (q4 + 1), :],
                                    in_=xt_d[0][:, 4 * q4:4 * (q4 + 1), :])
            cose_sb = singles.tile([128, S], F32, tag="cose")
            sine_sb = singles.tile([128, S], F32, tag="sine")
            nc.sync.dma_start(out=cose_sb[:], in_=cose_d[:])
            nc.sync.dma_start(out=sine_sb[:], in_=sine_d[:])
            nc.sync.dma_start(out=wk_sb[:], in_=wk_d[:])
            nc.sync.dma_start(out=wv_sb[:], in_=wv_d[:])
            nc.sync.dma_start(out=wo_sb[:], in_=wo_d[:])
            from concourse.masks import make_identity
            ones_sb = singles.tile([128, 128], BF, tag="ones")
            nc.vector.memset(ones_sb[:], 1.0)
            ident_sb = singles.tile([128, 128], BF, tag="ident")
            make_identity(nc, ident_sb[:])
            ktrot = singles.tile([128, H_LOC, S], BF, tag="ktrot")
            v_sb = singles.tile([128, NJ * H_LOC, SC], BF, tag="v")
            qtrot_all = singles.tile([128, H_LOC, S], BF, tag="qtrot_all")

            def projections(j, qdest, qsl, xt_pre=None):
                js = j * SC
                if xt_pre is not None:
                    xt_sb = xt_pre
                else:
                    xt_sb = xt_pool.tile([128, DCH, SC], BF, tag="xt")
                    nc.gpsimd.dma_start(out=xt_sb[:], in_=xt_d[j])

                for w_sb, dest, dsl in ((wq_sb, qdest, qsl),
                                        (wk_sb, ktrot, slice(js, js + SC))):
                    for hp in range(2):
                        parts = []
                        for hh in range(2):
                            h = hp * 2 + hh
                            if j == 0 and hp == 1:
                                ps = ps1.tile([128, SC], F32,
                                              tag="pv" if hh else "sums")
                            else:
                                ps = ps2.tile([128, SC], F32, tag="qkv1")
                            for d in range(DCH):
                                nc.tensor.matmul(
                                    ps[:],
                                    w_sb[:, d, h * 128:(h + 1) * 128],
                                    xt_sb[:, d, :],
                                    start=(d == 0), stop=(d == DCH - 1),
                                )
                            a_sb = rope_pool.tile([128, SC], F32, tag="ropeA")
                            nc.vector.tensor_mul(
                                a_sb[:], ps[:], cose_sb[:, js:js + SC])
                            b_sb = triples.tile([128, SC], F32, tag="ropeB")
                            nc.vector.tensor_mul(
                                b_sb[:], ps[:], sine_sb[:, js:js + SC])
                            b2_sb = triples.tile([128, SC], F32, tag="ropeB2")
                            nc.scalar.dma_start(out=b2_sb[0:64, :],
                                                in_=b_sb[64:128, :])
                            nc.scalar.dma_start(out=b2_sb[64:128, :],
                                                in_=b_sb[0:64, :])
                            parts.append((h, a_sb, b2_sb))
                        for h, a_sb, b2_sb in parts:
                            if dsl is None:
                                dst = dest[:, h, :]
                            else:
                                dst = dest[:, h, dsl]
                            nc.vector.tensor_add(dst, a_sb[:], b2_sb[:])

                for tl in range(4):
                    ps = ps2.tile([128, SC], F32, tag="qkv1")
                    for d in range(DCH):
                        nc.tensor.matmul(
                            ps[:],
                            xt_sb[:, d, tl * 128:(tl + 1) * 128],
                            wv_sb[:, d, :],
                            start=(d == 0), stop=(d == DCH - 1),
                        )
                    nc.scalar.copy(out=v_sb[:, 4 * j + tl, :], in_=ps[:])

            def wo_units(j, attnT_j, st, dcs):
                for dc in dcs:
                    wps = ps2.tile([128, SC], F32, tag="qkv1")
                    for h2 in range(H_LOC):
                        nc.tensor.matmul(
                            wps[:],
                            attnT_j[:, h2, st * 128:(st + 1) * 128],
                            wo_sb[:, h2, dc * SC:(dc + 1) * SC],
                            start=(h2 == 0), stop=(h2 == H_LOC - 1),
                        )
                    o_sb = stage_pool.tile([128, SC], F32, tag="ostage")
                    if (st + dc) % 2 == 0:
                        nc.scalar.copy(out=o_sb[:], in_=wps[:])
                    else:
                        nc.vector.tensor_copy(o_sb[:], wps[:])
                    nc.sync.dma_start(
                        out=out_v[4 * j + st][:, dc * SC:(dc + 1) * SC],
                        in_=o_sb[:])

            def attention_and_wo(j, qtrot_h, prev=None):
                maskt_sb = None
                if use_mask:
                    maskt_sb = xt_pool.tile([128, DCH, SC], BF, tag="xt")
                    nc.sync.dma_start(out=maskt_sb[:], in_=maskt_d[j])

                attnT_j = doubles.tile([128, H_LOC, SC], BF, tag="attnT")
                pg_order = list(range(DCH // 2))
                for h in range(H_LOC):
                    sums_ps = ps1.tile([128, SC], F32, tag="sums")
                    pv_ps = ps1.tile([128, SC], F32, tag="pv")
                    for gi, pg in enumerate(pg_order):
                        sc_ps = ps2.tile([128, 2, SC], F32, tag="sc")
                        exp_sb = stage_pool.tile([128, 2, SC], BF, tag="exp")
                        for i_ in range(2):
                            tt = pg * 2 + i_
                            nc.tensor.matmul(
                                sc_ps[:, i_, :],
                                ktrot[:, h, tt * 128:(tt + 1) * 128],
                                qtrot_h(h),
                                start=True, stop=not use_mask,
                            )
                            if use_mask:
                                nc.tensor.matmul(
                                    sc_ps[:, i_, :], ident_sb[:],
                                    maskt_sb[:, tt, :],
                                    start=False, stop=True,
                                )
                        nc.scalar.activation(out=exp_sb[:], in_=sc_ps[:],
                                             func=AF.Exp, scale=SCALE)
                        epair = epair_pool.tile([128, SC], BF, tag="epair")
                        nc.vector.tensor_add(epair[:], exp_sb[:, 0, :],
                                             exp_sb[:, 1, :])
                        nc.tensor.matmul(sums_ps[:], ones_sb[:], epair[:],
                                         start=gi == 0,
                                         stop=gi == len(pg_order) - 1)
                        for i_ in range(2):
                            tt = pg * 2 + i_
                            first = gi == 0 and i_ == 0
                            last = gi == len(pg_order) - 1 and i_ == 1
                            nc.tensor.matmul(pv_ps[:],
                                             v_sb[:, tt, h * 128:(h + 1) * 128],
                                             exp_sb[:, i_, :],
                                             start=first, stop=last)
                    recip_sb = doubles.tile([128, SC], F32, tag="recip")
                    nc.vector.reciprocal_approx_fast(out=recip_sb[:], in_=sums_ps[:])
                    nc.vector.tensor_mul(attnT_j[:, h, :], pv_ps[:], recip_sb[:])
                    if prev is not None:
                        wo_units(j - 1, prev, h, range(4))
                return attnT_j

            pending = None
            for j in range(nj):
                projections(j, qtrot_all, slice(j * SC, (j + 1) * SC),
                            xt_pre=xt0_sb if j == 0 else None)
            for j in range(nj):
                js = j * SC
                pending = attention_and_wo(
                    j, lambda h, js=js: qtrot_all[:, h, js:js + SC],
                    prev=pending)
            for st in range(4):
                wo_units(nj - 1, pending, st, range(4))

    nc.compile()
    return nc


def _get_build(variant):
    if variant not in _BUILDS:
        if variant == "causal":
            _BUILDS[variant] = _build_causal()
        else:
            _BUILDS[variant] = _build_legacy(variant)
    return _BUILDS[variant]


def _classify_mask(mask):
    if not np.any(mask):
        return "full_nomask"
    tril = np.tril(np.ones((S, S), dtype=bool))
    if np.all(mask[tril] == 0.0) and np.all(mask[~tril] <= -1e9):
        return "causal"
    return "full_mask"


def kernel(x, wq, wk, wv, wo, freqs_cos, freqs_sin, mask):
    global LAST_RESULT
    x = np.asarray(x, dtype=np.float32)
    wq, wk, wv, wo = (np.asarray(w, dtype=np.float32)
                      for w in (wq, wk, wv, wo))
    freqs_cos = np.asarray(freqs_cos, dtype=np.float32)
    freqs_sin = np.asarray(freqs_sin, dtype=np.float32)
    mask = np.asarray(mask, dtype=np.float32)

    variant = _classify_mask(mask)
    nc = _get_build(variant)

    if variant == "causal":
        in_maps = _host_inputs_causal(x, wq, wk, wv, wo, freqs_cos, freqs_sin)
    else:
        in_maps = _host_inputs_legacy(x, wq, wk, wv, wo, freqs_cos,
                                      freqs_sin, mask, variant)

    res = run_bass_kernel_spmd(nc, in_maps, list(range(N_CORES)))
    LAST_RESULT = res
    outs = [res.results[c]["out"].astype(np.float32) for c in range(N_CORES)]
    out = np.stack([
        outs[0] + outs[1] + outs[2] + outs[3],
        outs[4] + outs[5] + outs[6] + outs[7],
    ]).astype(np.float32)
    return out


def _host_inputs_legacy(x, wq, wk, wv, wo, freqs_cos, freqs_sin, mask,
                        variant):
    perm = np.concatenate([np.arange(0, 128, 2), np.arange(1, 128, 2)])

    def wproj_arr(w, g):
        cols = w[:, 512 * g:512 * (g + 1)].reshape(D, H_LOC, 128)
        cols = cols[:, :, perm].reshape(D, N_LOC)
        return np.ascontiguousarray(
            cols.reshape(DCH, 128, N_LOC).transpose(1, 0, 2)).astype(BF16)

    def wv_arr(w, g):
        cols = w[:, 512 * g:512 * (g + 1)]
        return np.ascontiguousarray(
            cols.reshape(DCH, 128, N_LOC).transpose(1, 0, 2)).astype(BF16)

    def wo_arr(g):
        rows = wo[512 * g:512 * (g + 1), :]
        return np.ascontiguousarray(
            rows.reshape(H_LOC, 128, D).transpose(1, 0, 2)).astype(BF16)

    cosE = np.empty((128, S), np.float32)
    sinE = np.empty((128, S), np.float32)
    cosE[0:64] = freqs_cos.T
    cosE[64:128] = freqs_cos.T
    sinE[0:64] = freqs_sin.T
    sinE[64:128] = -freqs_sin.T

    xt_b = []
    for b in range(B):
        xT = x[b].T.astype(BF16)
        xt = np.ascontiguousarray(
            xT.reshape(DCH, 128, NJ, SC).transpose(2, 1, 0, 3))
        xt_b.append(xt)

    maskt = None
    if variant == "full_mask":
        mT = (mask.T / SCALE).astype(BF16)
        maskt = np.ascontiguousarray(
            mT.reshape(DCH, 128, NJ, SC).transpose(2, 1, 0, 3))

    wq_g = [wproj_arr(wq, g) for g in range(H_LOC)]
    wk_g = [wproj_arr(wk, g) for g in range(H_LOC)]
    wv_g = [wv_arr(wv, g) for g in range(H_LOC)]
    wo_g = [wo_arr(g) for g in range(H_LOC)]

    in_maps = []
    for c in range(N_CORES):
        b, g = c // 4, c % 4
        m = {
            "xt": xt_b[b],
            "wq": wq_g[g], "wk": wk_g[g], "wv": wv_g[g], "wo": wo_g[g],
            "cose": cosE, "sine": sinE,
        }
        if maskt is not None:
            m["maskt"] = maskt
        in_maps.append(m)
    return in_maps


# revision 46
# speedup vs baseline: 1.1530x; 1.0124x over previous
"""Multi-head causal attention with RoPE on 8 Trainium2 NeuronCores.

Sharding: data-parallel over batch (2) x tensor-parallel over heads (16 -> 4
per core). Each core computes q/k/v projections for its 4 heads on its batch
element, attention, and a partial output projection (its rows of wo); the
host sums the 4 partials per batch element.

Device-side layout: everything is computed "transposed" (scores held as
[t, s]) so no on-device transposes are needed anywhere; softmax denominators
come from an all-ones matmul (partition reduction on the tensor engine).
RoPE pair-swap is done by permuting the wq/wk columns on the host into
(even|odd) half-layout so the swap becomes two partition-halved SBUF->SBUF
DMA copies.

Speed scheme (causal variant): hybrid precision. The first sequence chunk
(queries s<512 / keys t<512, where causal attention is concentrated and
quantization errors are amplified) runs in bf16; chunks 1-3 run their
q/k/v projections, attn@v, and output projection as fp8e4m3 DoubleRow
matmuls (2x tensor-engine throughput; two 128-deep K-tiles per pass).
Scores and softmax-denominator matmuls stay bf16 everywhere. The causal
diagonal mask is applied by zeroing exp tiles with affine_select on the
(otherwise idle) Pool engine instead of identity-matmul mask adds on the
PE. exp uses a -1 bias shift so fp8 exp tiles cannot overflow (max logit
~6.0 -> e^5.0 = 148 < 240); the shift cancels in the softmax ratio.
"""

import math

import ml_dtypes
import numpy as np

import concourse.bass as bass
import concourse.mybir as mybir
import concourse.tile as tile
from concourse import bacc
from concourse.bass_utils import run_bass_kernel_spmd

BF16 = ml_dtypes.bfloat16
FP8NP = ml_dtypes.float8_e4m3
F32 = mybir.dt.float32
BF = mybir.dt.bfloat16
FP8 = mybir.dt.float8e4
AF = mybir.ActivationFunctionType
DRMODE = mybir.MatmulPerfMode.DoubleRow

N_CORES = 8
B = 2
S = 2048
D = 2048
H = 16
HD = 128
H_LOC = 4          # heads per core
N_LOC = H_LOC * HD  # 512 local head dims
NJ = 4             # s-chunks
SC = S // NJ       # 512 s-chunk width
DCH = D // 128     # 16 contraction chunks
SCALE = 1.0 / math.sqrt(HD)
EXP_BIAS = -1.0    # exp(SCALE*score - 1): keeps fp8 exp tiles under 240

_BUILDS: dict = {}
LAST_RESULT = None


def _build_causal(nj: int = NJ):
    nc = bacc.Bacc("TRN2", target_bir_lowering=False, debug=False,
                   num_devices=N_CORES)

    # chunk 0 inputs (bf16)
    xt0_d = nc.dram_tensor("xt0", [128, DCH, SC], BF, kind="ExternalInput").ap()
    wqkb_d = nc.dram_tensor("wqkb", [2, H_LOC, 128, DCH, 128], BF,
                            kind="ExternalInput").ap()  # head-major q|k bf16
    wvb_d = nc.dram_tensor("wvb", [128, DCH, N_LOC], BF, kind="ExternalInput").ap()
    wob_d = nc.dram_tensor("wob", [128, H_LOC, D], BF, kind="ExternalInput").ap()
    # chunks 1-3 inputs (fp8)
    xt8_d = nc.dram_tensor("xt8", [NJ - 1, 128, DCH, SC], FP8,
                           kind="ExternalInput").ap()
    wq8_d = nc.dram_tensor("wq8", [128, DCH, N_LOC], FP8, kind="ExternalInput").ap()
    wk8_d = nc.dram_tensor("wk8", [128, DCH, N_LOC], FP8, kind="ExternalInput").ap()
    wv8_d = nc.dram_tensor("wv8", [128, DCH, N_LOC], FP8, kind="ExternalInput").ap()
    wo8_d = nc.dram_tensor("wo8", [128, H_LOC, D], FP8, kind="ExternalInput").ap()
    cose_d = nc.dram_tensor("cose", [128, S], F32, kind="ExternalInput").ap()
    sine_d = nc.dram_tensor("sine", [128, S], F32, kind="ExternalInput").ap()
    out_d = nc.dram_tensor("out", [S, D], BF, kind="ExternalOutput").ap()
    out_v = out_d.rearrange("(a p) d -> a p d", p=128)

    with tile.TileContext(nc) as tc:
        with (
            tc.tile_pool(name="singles", bufs=1) as singles,
            tc.tile_pool(name="doubles", bufs=2) as doubles,
            tc.tile_pool(name="triples", bufs=3) as triples,
            tc.tile_pool(name="quads", bufs=4) as quads,
            tc.tile_pool(name="hexes", bufs=7) as hexes,
            tc.tile_pool(name="ps1", bufs=1, space="PSUM") as ps1,
            tc.tile_pool(name="ps2", bufs=2, space="PSUM") as ps2,
        ):
            # ---- persistent tensors ----
            # chunk-0 q/k weights are streamed per head from wqkb_d; wv/wo
            # bf16 stay resident (v uses full-width rhs; wo reused per dc).
            wvb_sb = singles.tile([128, DCH, N_LOC], BF, tag="wvb")
            wob_sb = singles.tile([128, H_LOC, D], BF, tag="wob")
            wq8_sb = singles.tile([128, DCH, N_LOC], FP8, tag="wq8")
            wk8_sb = singles.tile([128, DCH, N_LOC], FP8, tag="wk8")
            wv8_sb = singles.tile([128, DCH, N_LOC], FP8, tag="wv8")
            wo8_sb = singles.tile([128, H_LOC, D], FP8, tag="wo8")
            xt0_sb = singles.tile([128, DCH, SC], BF, tag="xt0")
            # startup DMA order: xt0 pieces on the gpsimd queue, q/k weight
            # streams (piecewise) on sync in consumption order, cos/sin on
            # the scalar queue, later-needed tensors behind.
            for q4 in range(4):
                nc.gpsimd.dma_start(out=xt0_sb[:, 4 * q4:4 * (q4 + 1), :],
                                    in_=xt0_d[:, 4 * q4:4 * (q4 + 1), :])
            wqk_sb = {}
            for wi in range(2):
                for h in range(H_LOC):
                    wt_tile = triples.tile([128, DCH, 128], BF, tag="wstream")
                    for piece in range(2):
                        nc.sync.dma_start(
                            out=wt_tile[:, 8 * piece:8 * (piece + 1), :],
                            in_=wqkb_d[wi, h][:, 8 * piece:8 * (piece + 1), :])
                    wqk_sb[(wi, h)] = wt_tile
            cose_sb = singles.tile([128, S], F32, tag="cose")
            sine_sb = singles.tile([128, S], F32, tag="sine")
            nc.scalar.dma_start(out=cose_sb[:], in_=cose_d[:])
            nc.scalar.dma_start(out=sine_sb[:], in_=sine_d[:])
            nc.sync.dma_start(out=wvb_sb[:], in_=wvb_d[:])
            nc.sync.dma_start(out=wq8_sb[:], in_=wq8_d[:])
            nc.sync.dma_start(out=wk8_sb[:], in_=wk8_d[:])
            nc.sync.dma_start(out=wv8_sb[:], in_=wv8_d[:])
            nc.sync.dma_start(out=wob_sb[:], in_=wob_d[:])
            nc.sync.dma_start(out=wo8_sb[:], in_=wo8_d[:])
            ones_sb = singles.tile([128, 128], BF, tag="ones")
            nc.vector.memset(ones_sb[:], 1.0)
            bias_sb = singles.tile([128, 1], F32, tag="bias")
            nc.vector.memset(bias_sb[:], EXP_BIAS)
            # k^T (rotated, bf16) accumulates across chunks; v in fp8 for
            # DR pv (all chunks) + bf16 copy of t-chunk 0 for chunk-0 pv
            ktrot = singles.tile([128, H_LOC, S], BF, tag="ktrot")
            v8_sb = singles.tile([128, NJ * H_LOC, SC], FP8, tag="v8")
            vbf_sb = singles.tile([128, H_LOC, SC], BF, tag="vbf")

            def qk_unit(j, xt_sb, wi, hp, qdest):
                """DR/bf16 projections + rope for heads hp*2, hp*2+1 of
                q (wi=0) or k (wi=1) of chunk j."""
                js = j * SC
                dest, dsl = ((qdest, None) if wi == 0 else
                             (ktrot, slice(js, js + SC)))
                w8_sb = wq8_sb if wi == 0 else wk8_sb
                parts = []
                for hh in range(2):
                    h = hp * 2 + hh
                    if j == 0 and hp == 1:
                        ps = ps1.tile([128, SC], F32,
                                      tag="pv" if hh else "sums")
                    else:
                        ps = ps2.tile([128, SC], F32, tag="qkv1")
                    if j == 0:
                        wt = wqk_sb[(wi, h)]
                        for d in range(DCH):
                            nc.tensor.matmul(
                                ps[:], wt[:, d, :], xt_sb[:, d, :],
                                start=(d == 0), stop=(d == DCH - 1),
                            )
                    else:
                        for d in range(DCH // 2):
                            nc.tensor.matmul(
                                ps[:],
                                w8_sb[:, 2 * d:2 * d + 2,
                                      h * 128:(h + 1) * 128],
                                xt_sb[:, 2 * d:2 * d + 2, :],
                                start=(d == 0),
                                stop=(d == DCH // 2 - 1),
                                perf_mode=DRMODE,
                            )
                    a_sb = quads.tile([128, SC], F32, tag="ropeA")
                    nc.vector.tensor_mul(
                        a_sb[:], ps[:], cose_sb[:, js:js + SC])
                    b_sb = triples.tile([128, SC], F32, tag="ropeB")
                    nc.vector.tensor_mul(
                        b_sb[:], ps[:], sine_sb[:, js:js + SC])
                    # half-swap via SBUF->SBUF DMA (cross-partition)
                    b2_sb = triples.tile([128, SC], F32, tag="ropeB2")
                    nc.scalar.dma_start(out=b2_sb[0:64, :],
                                        in_=b_sb[64:128, :])
                    nc.scalar.dma_start(out=b2_sb[64:128, :],
                                        in_=b_sb[0:64, :])
                    parts.append((h, a_sb, b2_sb))
                for h, a_sb, b2_sb in parts:
                    if dsl is None:
                        dst = dest[:, h, :]
                    else:
                        dst = dest[:, h, dsl]
                    nc.vector.tensor_add(dst, a_sb[:], b2_sb[:])

            def v_unit(j, xt_sb, tl):
                ps = ps2.tile([128, SC], F32, tag="qkv1")
                if j == 0:
                    for d in range(DCH):
                        nc.tensor.matmul(
                            ps[:],
                            xt_sb[:, d, tl * 128:(tl + 1) * 128],
                            wvb_sb[:, d, :],
                            start=(d == 0), stop=(d == DCH - 1),
                        )
                    nc.scalar.copy(out=vbf_sb[:, tl, :], in_=ps[:])
                    # pool can't read PSUM; mirror to fp8 from the SBUF copy
                    nc.gpsimd.tensor_copy(v8_sb[:, tl, :], vbf_sb[:, tl, :])
                else:
                    for d in range(DCH // 2):
                        nc.tensor.matmul(
                            ps[:],
                            xt_sb[:, 2 * d:2 * d + 2,
                                  tl * 128:(tl + 1) * 128],
                            wv8_sb[:, 2 * d:2 * d + 2, :],
                            start=(d == 0), stop=(d == DCH // 2 - 1),
                            perf_mode=DRMODE,
                        )
                    nc.scalar.copy(out=v8_sb[:, 4 * j + tl, :], in_=ps[:])

            def projection_units(j, qdest):
                """Emit the xt DMA now; return per-unit closures for the
                matmul/rope work (interleaved into the previous chunk's
                attention as PE bubble fillers)."""
                # sync queue (idle after startup): the gpsimd queue would
                # serialize these triggers behind pool-engine compute
                xt_sb = doubles.tile([128, DCH, SC], FP8, tag="xt8")
                for q4 in range(4):
                    nc.sync.dma_start(
                        out=xt_sb[:, 4 * q4:4 * (q4 + 1), :],
                        in_=xt8_d[j - 1][:, 4 * q4:4 * (q4 + 1), :])
                units = []
                for wi in range(2):
                    for hp in range(2):
                        units.append(lambda wi=wi, hp=hp:
                                     qk_unit(j, xt_sb, wi, hp, qdest))
                for tl in range(4):
                    units.append(lambda tl=tl: v_unit(j, xt_sb, tl))
                return units

            def projections0(qdest):
                for wi in range(2):
                    for hp in range(2):
                        qk_unit(0, xt0_sb, wi, hp, qdest)
                for tl in range(4):
                    v_unit(0, xt0_sb, tl)

            def wo_units(j, attnT_j, st, dcs):
                for dc in dcs:
                    wps = ps2.tile([128, SC], F32, tag="qkv1")
                    if j == 0:
                        for h2 in range(H_LOC):
                            nc.tensor.matmul(
                                wps[:],
                                attnT_j[:, h2, st * 128:(st + 1) * 128],
                                wob_sb[:, h2, dc * SC:(dc + 1) * SC],
                                start=(h2 == 0), stop=(h2 == H_LOC - 1),
                            )
                    else:
                        for hp in range(H_LOC // 2):
                            nc.tensor.matmul(
                                wps[:],
                                attnT_j[:, 2 * hp:2 * hp + 2,
                                        st * 128:(st + 1) * 128],
                                wo8_sb[:, 2 * hp:2 * hp + 2,
                                       dc * SC:(dc + 1) * SC],
                                start=(hp == 0), stop=(hp == H_LOC // 2 - 1),
                                perf_mode=DRMODE,
                            )
                    o_sb = triples.tile([128, SC], BF, tag="ostage")
                    if (st + dc) % 2 == 0:
                        nc.scalar.copy(out=o_sb[:], in_=wps[:])
                    else:
                        nc.vector.tensor_copy(o_sb[:], wps[:])
                    nc.sync.dma_start(
                        out=out_v[4 * j + st][:, dc * SC:(dc + 1) * SC],
                        in_=o_sb[:])

            def attention_and_wo(j, qtrot, fillers=(), self_wo=False):
                fillers = list(fillers)
                emitted = [0]

                def pump(frac):
                    # keep the in-order PE queue fed: emit filler units up to
                    # the given fraction of attention progress
                    n_emit = min(len(fillers),
                                 int(len(fillers) * frac + 0.999))
                    while emitted[0] < n_emit:
                        fillers[emitted[0]]()
                        emitted[0] += 1

                bf = j == 0
                edt = BF if bf else FP8
                if bf:
                    attnT_j = singles.tile([128, H_LOC, SC], BF, tag="attnT_bf")
                else:
                    attnT_j = doubles.tile([128, H_LOC, SC], FP8, tag="attnT8")
                # diagonal pair-groups first: their longer select+exp chain
                # overlaps the remaining full groups' matmuls
                pg_order = list(range(2 * j, 2 * j + 2)) + list(range(2 * j))
                ng = len(pg_order)
                for h in range(H_LOC):
                    sums_ps = ps1.tile([128, SC], F32, tag="sums")
                    pv_ps = ps1.tile([128, SC], F32, tag="pv")

                    def emit_scores_exp(gi):
                        """scores + exp/select/epair chain for group gi;
                        returns (exp_sb, equad-or-None)."""
                        pg = pg_order[gi]
                        sc_ps = ps2.tile([128, 2, SC], F32, tag="sc")
                        if bf:
                            exp_sb = doubles.tile([128, 2, SC], BF,
                                                  tag="exp_bf")
                        else:
                            exp_sb = triples.tile([128, 2, SC], FP8, tag="exp8")
                        diag = pg >= 2 * j
                        for i_ in range(2):
                            tt = pg * 2 + i_
                            w0 = 128 * (tt - 4 * j) if diag else 0
                            nc.tensor.matmul(
                                sc_ps[:, i_, w0:SC],
                                ktrot[:, h, tt * 128:(tt + 1) * 128],
                                qtrot[:, h, w0:SC],
                                start=True, stop=True,
                            )
                        if diag:
                            # columns [0, 128p) of a diagonal block are fully
                            # causal-masked: skip their scores/exp, zero-fill
                            # via dependency-free memset; select covers only
                            # the block's own 128-wide partial triangle
                            for i_ in range(2):
                                p = pg * 2 + i_ - 4 * j
                                if p > 0:
                                    nc.gpsimd.memset(
                                        exp_sb[:, i_, 0:128 * p], 0.0)
                                nc.scalar.activation(
                                    out=exp_sb[:, i_, 128 * p:SC],
                                    in_=sc_ps[:, i_, 128 * p:SC],
                                    func=AF.Exp, scale=SCALE, bias=bias_sb[:])
                                nc.gpsimd.affine_select(
                                    exp_sb[:, i_, 128 * p:128 * (p + 1)],
                                    exp_sb[:, i_, 128 * p:128 * (p + 1)],
                                    pattern=[[1, 128]],
                                    compare_op=mybir.AluOpType.is_ge,
                                    fill=0.0, base=0,
                                    channel_multiplier=-1)
                        else:
                            nc.scalar.activation(out=exp_sb[:], in_=sc_ps[:],
                                                 func=AF.Exp, scale=SCALE,
                                                 bias=bias_sb[:])
                        epair = doubles.tile([128, SC], BF, tag="epair")
                        nc.vector.tensor_add(epair[:], exp_sb[:, 0, :],
                                             exp_sb[:, 1, :])
                        equad = None
                        if gi % 2 == 0:
                            pend_epair[0] = epair
                        else:
                            equad = triples.tile([128, SC], BF, tag="equad")
                            nc.vector.tensor_add(equad[:], pend_epair[0][:],
                                                 epair[:])
                        return exp_sb, equad

                    def emit_pe_consumers(gi, exp_sb, equad):
                        pg = pg_order[gi]
                        if equad is not None:
                            nc.tensor.matmul(sums_ps[:], ones_sb[:], equad[:],
                                             start=gi == 1, stop=gi == ng - 1)
                        if bf:
                            for i_ in range(2):
                                tt = pg * 2 + i_
                                nc.tensor.matmul(
                                    pv_ps[:],
                                    vbf_sb[:, tt, h * 128:(h + 1) * 128],
                                    exp_sb[:, i_, :],
                                    start=gi == 0 and i_ == 0,
                                    stop=gi == ng - 1 and i_ == 1)
                        else:
                            nc.tensor.matmul(
                                pv_ps[:],
                                v8_sb[:, 2 * pg:2 * pg + 2,
                                      h * 128:(h + 1) * 128],
                                exp_sb[:, :, :],
                                start=gi == 0, stop=gi == ng - 1,
                                perf_mode=DRMODE)

                    # depth-1 software pipeline: group gi's scores run on the
                    # PE while group gi-1's exp/select chain completes, so the
                    # ones/pv consumers of gi-1 issue with their deps resolved
                    pend_epair = [None]
                    pending_grp = None
                    for gi in range(ng):
                        made = emit_scores_exp(gi)
                        if pending_grp is not None:
                            emit_pe_consumers(*pending_grp)
                        pending_grp = (gi, *made)
                    emit_pe_consumers(*pending_grp)
                    recip_sb = doubles.tile([128, SC], F32, tag="recip")
                    nc.vector.reciprocal_approx_fast(out=recip_sb[:],
                                                     in_=sums_ps[:])
                    nc.vector.tensor_mul(attnT_j[:, h, :], pv_ps[:],
                                         recip_sb[:])
                    pump((h + 1) / H_LOC if h < H_LOC - 1 else 1.0)
                return attnT_j

            # prev-chunk wo and next-chunk projections run interleaved with
            # each chunk's attention, keeping the in-order PE queue fed while
            # softmax chains (ACT exp -> Pool select -> DVE sums) resolve
            def wo_closures(j, attnT_j):
                out = []
                for st in range(4):
                    for dp in range(2):
                        out.append(lambda st=st, dp=dp: wo_units(
                            j, attnT_j, st, [2 * dp, 2 * dp + 1]))
                return out

            qtrot = doubles.tile([128, H_LOC, SC], BF, tag="qtrot")
            projections0(qtrot)
            pending = None
            for j in range(nj):
                fillers = []
                punits = []
                qtrot_next = None
                if j + 1 < nj:
                    qtrot_next = doubles.tile([128, H_LOC, SC], BF,
                                              tag="qtrot")
                    punits = projection_units(j + 1, qtrot_next)
                wunits = wo_closures(j - 1, pending) if pending is not None \
                    else []
                # interleave so projection DVE bursts spread across heads
                for a, b in zip(wunits + [None] * len(punits),
                                punits + [None] * len(wunits)):
                    if a is not None:
                        fillers.append(a)
                    if b is not None:
                        fillers.append(b)
                pending = attention_and_wo(j, qtrot, fillers)
                qtrot = qtrot_next
            for st in range(4):
                wo_units(nj - 1, pending, st, range(4))

    nc.compile()
    return nc


def _host_inputs_causal(x, wq, wk, wv, wo, freqs_cos, freqs_sin):
    # half-layout column permutation within each head (even indices then odd)
    perm = np.concatenate([np.arange(0, 128, 2), np.arange(1, 128, 2)])

    def wproj_cols(w, g):
        cols = w[:, 512 * g:512 * (g + 1)].reshape(D, H_LOC, 128)
        return cols[:, :, perm].reshape(D, N_LOC)

    def as_dch(cols, dt):
        return np.ascontiguousarray(
            cols.reshape(DCH, 128, N_LOC).transpose(1, 0, 2)).astype(dt)

    def wqkb_arr(g):
        # [2(q|k), H_LOC, 128, DCH, 128] bf16, head-major for streaming
        out = np.empty((2, H_LOC, 128, DCH, 128), BF16)
        for wi, w in enumerate((wq, wk)):
            cols = wproj_cols(w, g).reshape(DCH, 128, H_LOC, 128)
            out[wi] = cols.transpose(2, 1, 0, 3).astype(BF16)
        return out

    def wv_arr(g, dt):
        cols = wv[:, 512 * g:512 * (g + 1)]
        return as_dch(cols, dt)

    def wo_arr(g, dt):
        rows = wo[512 * g:512 * (g + 1), :]
        return np.ascontiguousarray(
            rows.reshape(H_LOC, 128, D).transpose(1, 0, 2)).astype(dt)

    # cos/sin in half-layout: rows j and j+64 carry pair j's cos; sine rows
    # 0..63 = +sin (source a_j -> target j+64), rows 64..127 = -sin
    cosE = np.empty((128, S), np.float32)
    sinE = np.empty((128, S), np.float32)
    cosE[0:64] = freqs_cos.T
    cosE[64:128] = freqs_cos.T
    sinE[0:64] = freqs_sin.T
    sinE[64:128] = -freqs_sin.T

    xt0_b, xt8_b = [], []
    for b in range(B):
        xT = x[b].T  # [D, S] f32
        xt = xT.reshape(DCH, 128, NJ, SC).transpose(2, 1, 0, 3)
        xt0_b.append(np.ascontiguousarray(xt[0]).astype(BF16))
        xt8_b.append(np.ascontiguousarray(xt[1:]).astype(FP8NP))

    in_maps = []
    for c in range(N_CORES):
        b, g = c // 4, c % 4
        m = {
            "xt0": xt0_b[b], "xt8": xt8_b[b],
            "wqkb": wqkb_arr(g),
            "wvb": wv_arr(g, BF16), "wob": wo_arr(g, BF16),
            "wq8": as_dch(wproj_cols(wq, g), FP8NP),
            "wk8": as_dch(wproj_cols(wk, g), FP8NP),
            "wv8": wv_arr(g, FP8NP), "wo8": wo_arr(g, FP8NP),
            "cose": cosE, "sine": sinE,
        }
        in_maps.append(m)
    return in_maps


# ---------------------------------------------------------------------------
# legacy bf16 build for the non-causal variants (full attention / arbitrary
# additive mask) -- unchanged from the baseline implementation
# ---------------------------------------------------------------------------
def _build_legacy(variant: str, nj: int = NJ):
    use_mask = variant == "full_mask"

    nc = bacc.Bacc("TRN2", target_bir_lowering=False, debug=False,
                   num_devices=N_CORES)

    xt_d = nc.dram_tensor("xt", [NJ, 128, DCH, SC], BF, kind="ExternalInput").ap()
    wq_d = nc.dram_tensor("wq", [128, DCH, N_LOC], BF, kind="ExternalInput").ap()
    wk_d = nc.dram_tensor("wk", [128, DCH, N_LOC], BF, kind="ExternalInput").ap()
    wv_d = nc.dram_tensor("wv", [128, DCH, N_LOC], BF, kind="ExternalInput").ap()
    wo_d = nc.dram_tensor("wo", [128, H_LOC, D], BF, kind="ExternalInput").ap()
    cose_d = nc.dram_tensor("cose", [128, S], F32, kind="ExternalInput").ap()
    sine_d = nc.dram_tensor("sine", [128, S], F32, kind="ExternalInput").ap()
    maskt_d = None
    if use_mask:
        maskt_d = nc.dram_tensor("maskt", [NJ, 128, DCH, SC], BF,
                                 kind="ExternalInput").ap()
    out_d = nc.dram_tensor("out", [S, D], F32, kind="ExternalOutput").ap()
    out_v = out_d.rearrange("(a p) d -> a p d", p=128)

    with tile.TileContext(nc) as tc:
        with (
            tc.tile_pool(name="singles", bufs=1) as singles,
            tc.tile_pool(name="doubles", bufs=2) as doubles,
            tc.tile_pool(name="triples", bufs=3) as triples,
            tc.tile_pool(name="ps1", bufs=1, space="PSUM") as ps1,
            tc.tile_pool(name="ps2", bufs=2, space="PSUM") as ps2,
        ):
            rope_pool = doubles if use_mask else triples
            stage_pool = doubles if use_mask else triples
            epair_pool = doubles
            wq_sb = singles.tile([128, DCH, N_LOC], BF, tag="wq")
            wk_sb = singles.tile([128, DCH, N_LOC], BF, tag="wk")
            wv_sb = singles.tile([128, DCH, N_LOC], BF, tag="wv")
            wo_sb = singles.tile([128, H_LOC, D], BF, tag="wo")
            xt_pool = singles if use_mask else doubles
            xt0_sb = xt_pool.tile([128, DCH, SC], BF, tag="xt")
            for q4 in range(4):
                nc.sync.dma_start(out=wq_sb[:, 4 * q4:4 * (q4 + 1), :],
                                  in_=wq_d[:, 4 * q4:4 * (q4 + 1), :])
                nc.gpsimd.dma_start(out=xt0_sb[:, 4 * q4:4 * (q4 + 1), :],
                                    in_=xt_d[0][:, 4 * q4:4 * (q4 + 1), :])
            cose_sb = singles.tile([128, S], F32, tag="cose")
            sine_sb = singles.tile([128, S], F32, tag="sine")
            nc.sync.dma_start(out=cose_sb[:], in_=cose_d[:])
            nc.sync.dma_start(out=sine_sb[:], in_=sine_d[:])
            nc.sync.dma_start(out=wk_sb[:], in_=wk_d[:])
            nc.sync.dma_start(out=wv_sb[:], in_=wv_d[:])
            nc.sync.dma_start(out=wo_sb[:], in_=wo_d[:])
            from concourse.masks import make_identity
            ones_sb = singles.tile([128, 128], BF, tag="ones")
            nc.vector.memset(ones_sb[:], 1.0)
            ident_sb = singles.tile([128, 128], BF, tag="ident")
            make_identity(nc, ident_sb[:])
            ktrot = singles.tile([128, H_LOC, S], BF, tag="ktrot")
            v_sb = singles.tile([128, NJ * H_LOC, SC], BF, tag="v")
            qtrot_all = singles.tile([128, H_LOC, S], BF, tag="qtrot_all")

            def projections(j, qdest, qsl, xt_pre=None):
                js = j * SC
                if xt_pre is not None:
                    xt_sb = xt_pre
                else:
                    xt_sb = xt_pool.tile([128, DCH, SC], BF, tag="xt")
                    nc.gpsimd.dma_start(out=xt_sb[:], in_=xt_d[j])

                for w_sb, dest, dsl in ((wq_sb, qdest, qsl),
                                        (wk_sb, ktrot, slice(js, js + SC))):
                    for hp in range(2):
                        parts = []
                        for hh in range(2):
                            h = hp * 2 + hh
                            if j == 0 and hp == 1:
                                ps = ps1.tile([128, SC], F32,
                                              tag="pv" if hh else "sums")
                            else:
                                ps = ps2.tile([128, SC], F32, tag="qkv1")
                            for d in range(DCH):
                                nc.tensor.matmul(
                                    ps[:],
                                    w_sb[:, d, h * 128:(h + 1) * 128],
                                    xt_sb[:, d, :],
                                    start=(d == 0), stop=(d == DCH - 1),
                                )
                            a_sb = rope_pool.tile([128, SC], F32, tag="ropeA")
                            nc.vector.tensor_mul(
                                a_sb[:], ps[:], cose_sb[:, js:js + SC])
                            b_sb = triples.tile([128, SC], F32, tag="ropeB")
                            nc.vector.tensor_mul(
                                b_sb[:], ps[:], sine_sb[:, js:js + SC])
                            b2_sb = triples.tile([128, SC], F32, tag="ropeB2")
                            nc.scalar.dma_start(out=b2_sb[0:64, :],
                                                in_=b_sb[64:128, :])
                            nc.scalar.dma_start(out=b2_sb[64:128, :],
                                                in_=b_sb[0:64, :])
                            parts.append((h, a_sb, b2_sb))
                        for h, a_sb, b2_sb in parts:
                            if dsl is None:
                                dst = dest[:, h, :]
                            else:
                                dst = dest[:, h, dsl]
                            nc.vector.tensor_add(dst, a_sb[:], b2_sb[:])

                for tl in range(4):
                    ps = ps2.tile([128, SC], F32, tag="qkv1")
                    for d in range(DCH):
                        nc.tensor.matmul(
                            ps[:],
                            xt_sb[:, d, tl * 128:(tl + 1) * 128],
                            wv_sb[:, d, :],
                            start=(d == 0), stop=(d == DCH - 1),
                        )
                    nc.scalar.copy(out=v_sb[:, 4 * j + tl, :], in_=ps[:])

            def wo_units(j, attnT_j, st, dcs):
                for dc in dcs:
                    wps = ps2.tile([128, SC], F32, tag="qkv1")
                    for h2 in range(H_LOC):
                        nc.tensor.matmul(
                            wps[:],
                            attnT_j[:, h2, st * 128:(st + 1) * 128],
                            wo_sb[:, h2, dc * SC:(dc + 1) * SC],
                            start=(h2 == 0), stop=(h2 == H_LOC - 1),
                        )
                    o_sb = stage_pool.tile([128, SC], F32, tag="ostage")
                    if (st + dc) % 2 == 0:
                        nc.scalar.copy(out=o_sb[:], in_=wps[:])
                    else:
                        nc.vector.tensor_copy(o_sb[:], wps[:])
                    nc.sync.dma_start(
                        out=out_v[4 * j + st][:, dc * SC:(dc + 1) * SC],
                        in_=o_sb[:])

            def attention_and_wo(j, qtrot_h, prev=None):
                maskt_sb = None
                if use_mask:
                    maskt_sb = xt_pool.tile([128, DCH, SC], BF, tag="xt")
                    nc.sync.dma_start(out=maskt_sb[:], in_=maskt_d[j])

                attnT_j = doubles.tile([128, H_LOC, SC], BF, tag="attnT")
                pg_order = list(range(DCH // 2))
                for h in range(H_LOC):
                    sums_ps = ps1.tile([128, SC], F32, tag="sums")
                    pv_ps = ps1.tile([128, SC], F32, tag="pv")
                    for gi, pg in enumerate(pg_order):
                        sc_ps = ps2.tile([128, 2, SC], F32, tag="sc")
                        exp_sb = stage_pool.tile([128, 2, SC], BF, tag="exp")
                        for i_ in range(2):
                            tt = pg * 2 + i_
                            nc.tensor.matmul(
                                sc_ps[:, i_, :],
                                ktrot[:, h, tt * 128:(tt + 1) * 128],
                                qtrot_h(h),
                                start=True, stop=not use_mask,
                            )
                            if use_mask:
                                nc.tensor.matmul(
                                    sc_ps[:, i_, :], ident_sb[:],
                                    maskt_sb[:, tt, :],
                                    start=False, stop=True,
                                )
                        nc.scalar.activation(out=exp_sb[:], in_=sc_ps[:],
                                             func=AF.Exp, scale=SCALE)
                        epair = epair_pool.tile([128, SC], BF, tag="epair")
                        nc.vector.tensor_add(epair[:], exp_sb[:, 0, :],
                                             exp_sb[:, 1, :])
                        nc.tensor.matmul(sums_ps[:], ones_sb[:], epair[:],
                                         start=gi == 0,
                                         stop=gi == len(pg_order) - 1)
                        for i_ in range(2):
                            tt = pg * 2 + i_
                            first = gi == 0 and i_ == 0
                            last = gi == len(pg_order) - 1 and i_ == 1
                            nc.tensor.matmul(pv_ps[:],
                                             v_sb[:, tt, h * 128:(h + 1) * 128],
                                             exp_sb[:, i_, :],
                                             start=first, stop=last)
                    recip_sb = doubles.tile([128, SC], F32, tag="recip")
                    nc.vector.reciprocal_approx_fast(out=recip_sb[:], in_=sums_ps[:])
                    nc.vector.tensor_mul(attnT_j[:, h, :], pv_ps[:], recip_sb[:])
                    if prev is not None:
                        wo_units(j - 1, prev, h, range(4))
                return attnT_j

            pending = None
            for j in range(nj):
                projections(j, qtrot_all, slice(j * SC, (j + 1) * SC),
                            xt_pre=xt0_sb if j == 0 else None)
            for j in range(nj):
                js = j * SC
                pending = attention_and_wo(
                    j, lambda h, js=js: qtrot_all[:, h, js:js + SC],
                    prev=pending)
            for st in range(4):
                wo_units(nj - 1, pending, st, range(4))

    nc.compile()
    return nc


def _get_build(variant):
    if variant not in _BUILDS:
        if variant == "causal":
            _BUILDS[variant] = _build_causal()
        else:
            _BUILDS[variant] = _build_legacy(variant)
    return _BUILDS[variant]


def _classify_mask(mask):
    if not np.any(mask):
        return "full_nomask"
    tril = np.tril(np.ones((S, S), dtype=bool))
    if np.all(mask[tril] == 0.0) and np.all(mask[~tril] <= -1e9):
        return "causal"
    return "full_mask"


def kernel(x, wq, wk, wv, wo, freqs_cos, freqs_sin, mask):
    global LAST_RESULT
    x = np.asarray(x, dtype=np.float32)
    wq, wk, wv, wo = (np.asarray(w, dtype=np.float32)
                      for w in (wq, wk, wv, wo))
    freqs_cos = np.asarray(freqs_cos, dtype=np.float32)
    freqs_sin = np.asarray(freqs_sin, dtype=np.float32)
    mask = np.asarray(mask, dtype=np.float32)

    variant = _classify_mask(mask)
    nc = _get_build(variant)

    if variant == "causal":
        in_maps = _host_inputs_causal(x, wq, wk, wv, wo, freqs_cos, freqs_sin)
    else:
        in_maps = _host_inputs_legacy(x, wq, wk, wv, wo, freqs_cos,
                                      freqs_sin, mask, variant)

    res = run_bass_kernel_spmd(nc, in_maps, list(range(N_CORES)))
    LAST_RESULT = res
    outs = [res.results[c]["out"].astype(np.float32) for c in range(N_CORES)]
    out = np.stack([
        outs[0] + outs[1] + outs[2] + outs[3],
        outs[4] + outs[5] + outs[6] + outs[7],
    ]).astype(np.float32)
    return out


def _host_inputs_legacy(x, wq, wk, wv, wo, freqs_cos, freqs_sin, mask,
                        variant):
    perm = np.concatenate([np.arange(0, 128, 2), np.arange(1, 128, 2)])

    def wproj_arr(w, g):
        cols = w[:, 512 * g:512 * (g + 1)].reshape(D, H_LOC, 128)
        cols = cols[:, :, perm].reshape(D, N_LOC)
        return np.ascontiguousarray(
            cols.reshape(DCH, 128, N_LOC).transpose(1, 0, 2)).astype(BF16)

    def wv_arr(w, g):
        cols = w[:, 512 * g:512 * (g + 1)]
        return np.ascontiguousarray(
            cols.reshape(DCH, 128, N_LOC).transpose(1, 0, 2)).astype(BF16)

    def wo_arr(g):
        rows = wo[512 * g:512 * (g + 1), :]
        return np.ascontiguousarray(
            rows.reshape(H_LOC, 128, D).transpose(1, 0, 2)).astype(BF16)

    cosE = np.empty((128, S), np.float32)
    sinE = np.empty((128, S), np.float32)
    cosE[0:64] = freqs_cos.T
    cosE[64:128] = freqs_cos.T
    sinE[0:64] = freqs_sin.T
    sinE[64:128] = -freqs_sin.T

    xt_b = []
    for b in range(B):
        xT = x[b].T.astype(BF16)
        xt = np.ascontiguousarray(
            xT.reshape(DCH, 128, NJ, SC).transpose(2, 1, 0, 3))
        xt_b.append(xt)

    maskt = None
    if variant == "full_mask":
        mT = (mask.T / SCALE).astype(BF16)
        maskt = np.ascontiguousarray(
            mT.reshape(DCH, 128, NJ, SC).transpose(2, 1, 0, 3))

    wq_g = [wproj_arr(wq, g) for g in range(H_LOC)]
    wk_g = [wproj_arr(wk, g) for g in range(H_LOC)]
    wv_g = [wv_arr(wv, g) for g in range(H_LOC)]
    wo_g = [wo_arr(g) for g in range(H_LOC)]

    in_maps = []
    for c in range(N_CORES):
        b, g = c // 4, c % 4
        m = {
            "xt": xt_b[b],
            "wq": wq_g[g], "wk": wk_g[g], "wv": wv_g[g], "wo": wo_g[g],
            "cose": cosE, "sine": sinE,
        }
        if maskt is not None:
            m["maskt"] = maskt
        in_maps.append(m)
    return in_maps
